# revision 6
# baseline (speedup 1.0000x reference)
"""Trainium2 Bass kernel for causal multi-head attention (v2).

Problem: B=4, S=2048, D=1024, H=16 heads (d_head=64), fp32 I/O.
    qkv = x @ w_qkv + b_qkv ; causal softmax attention ; out @ w_out + b_out

Sharding over 8 NeuronCores: data-parallel over batch (4) x
tensor-parallel over head-groups (2 groups of 8 heads). Core c handles
batch c//2, head-group c%2. No collectives: each core returns its
partial out-projection; the host sums the two group partials per batch
and adds b_out.

v2 schedule (vs v1): the kernel is ACT(exp)-heavy in attention and
PE-heavy in the projections; v1 ran them serially, so the PE idled
waiting on exp in attention, HAM re-throttled it to half clock, and
the whole attention phase crawled.  v2 keeps one dense in-order PE
stream: attention starts as soon as head-pair 0's q/k m-tile is
projected, and all remaining projection work (qk m-tiles 1-3, v per
pair, out-projection) is held in a filler queue that is drained
between scores/AV groups.  The PE never waits on the ACT: every
dependency point has independent matmul work queued behind it.
ib=1 (queries 1024-2047) is processed first so the out-projection of
those rows becomes filler for the lighter ib=0 phase.

Layout notes: qT/kT = w.T @ xT (transposed acts, no on-device
transposes), v natural [s, h, 65] with a ones column per head so the
AV matmul also yields the softmax denominator; scoresT[j,i] per head
with K=64 — the two heads of an m-tile sit on partitions 0:64/64:128
so their scores matmuls land on different PE row-groups and run
concurrently (2x); max-free softmax on ScalarE (|logit| < ~7); causal
handled by narrowing to [c0, IB) plus one 128x128 triangular mask on
diagonal blocks; weights arrive as host-packed single-DMA tiles.
"""

import sys

if "/opt/trn_rl_repo" not in sys.path:
    sys.path.insert(0, "/opt/trn_rl_repo")

from collections import deque

import numpy as np
import ml_dtypes

B, S, D = 4, 2048, 1024
H, DH = 16, 64
G = 2                # tensor-parallel head groups
HPG = H // G         # heads per group (8)
CG = HPG * DH        # channel cols per group (512)
N_CORES = 8
BF16 = ml_dtypes.bfloat16

KT = D // 128        # 8 contraction k-tiles for the projections
IB = 1024            # i-block (query positions per attention block)
NIB = S // IB        # 2

_cache = {}


def _build_program():
    import concourse.tile as tile
    from concourse import bacc, mybir

    f32 = mybir.dt.float32
    bf16 = mybir.dt.bfloat16
    Exp = mybir.ActivationFunctionType.Exp
    Ident = mybir.ActivationFunctionType.Identity

    nc = bacc.Bacc("TRN2", target_bir_lowering=False, debug=False,
                   num_devices=N_CORES)

    xT_d = nc.dram_tensor("xT", [D, S], bf16, kind="ExternalInput").ap()
    # m-major pack: cols m*1024 + k*128 hold w[k-tile rows, m-tile cols]
    wq_d = nc.dram_tensor("wq", [128, 4096], bf16, kind="ExternalInput").ap()
    wk_d = nc.dram_tensor("wk", [128, 4096], bf16, kind="ExternalInput").ap()
    # k-major pack: cols k*512 hold w[k-tile rows, :]
    wv_d = nc.dram_tensor("wv", [128, 4096], bf16, kind="ExternalInput").ap()
    wo_d = nc.dram_tensor("wo", [128, 4096], bf16, kind="ExternalInput").ap()
    bqk_d = nc.dram_tensor("bqk", [128, 8], f32, kind="ExternalInput").ap()
    bv_d = nc.dram_tensor("bv", [1, CG], bf16, kind="ExternalInput").ap()
    tri_d = nc.dram_tensor("tri", [128, 128], bf16, kind="ExternalInput").ap()
    y_d = nc.dram_tensor("y", [S, D], f32, kind="ExternalOutput").ap()

    with tile.TileContext(nc) as tc:
        with (
            tc.tile_pool(name="consts", bufs=1) as cpool,
            tc.tile_pool(name="acts", bufs=1) as apool,
            tc.tile_pool(name="exps", bufs=12) as epool,
            tc.tile_pool(name="small", bufs=4) as spool,
            tc.tile_pool(name="rbc", bufs=2) as rpool,
            tc.tile_pool(name="ystage", bufs=3) as ypool,
            tc.tile_pool(name="psum_s", bufs=2, space="PSUM") as sp,
            tc.tile_pool(name="psum_av", bufs=2, space="PSUM") as avp,
        ):
            # ---- constants: few large DMAs, gating tiles first ----
            wq_sb = cpool.tile([128, 4096], bf16, tag="wq", name="wq_sb")
            wk_sb = cpool.tile([128, 4096], bf16, tag="wk", name="wk_sb")
            wv_sb = cpool.tile([128, 4096], bf16, tag="wv", name="wv_sb")
            wo_sb = cpool.tile([128, 4096], bf16, tag="wo", name="wo_sb")
            bqk = cpool.tile([128, 8], f32, tag="bqk", name="bqk")
            bv_row = cpool.tile([1, CG], bf16, tag="bv", name="bv_row")
            tri = cpool.tile([128, 128], bf16, tag="tri", name="tri")
            xt = [cpool.tile([128, S], bf16, tag=f"xt{k}", name=f"xt{k}")
                  for k in range(KT)]

            nc.sync.dma_start(wq_sb[:, 0:1024], wq_d[:, 0:1024])
            nc.sync.dma_start(xt[0][:], xT_d[0:128, :])
            nc.sync.dma_start(wk_sb[:, 0:1024], wk_d[:, 0:1024])
            nc.sync.dma_start(bqk[:], bqk_d[:])
            nc.sync.dma_start(bv_row[:], bv_d[:])
            nc.sync.dma_start(tri[:], tri_d[:])
            nc.sync.dma_start(xt[1][:], xT_d[128:256, :])
            nc.sync.dma_start(wv_sb[:], wv_d[:])
            for k in range(2, KT):
                nc.sync.dma_start(xt[k][:], xT_d[k * 128:(k + 1) * 128, :])
            nc.sync.dma_start(wq_sb[:, 1024:4096], wq_d[:, 1024:4096])
            nc.sync.dma_start(wk_sb[:, 1024:4096], wk_d[:, 1024:4096])
            nc.sync.dma_start(wo_sb[:], wo_d[:])

            ones_row = cpool.tile([1, 128], bf16, tag="ones", name="ones_row")
            nc.gpsimd.memset(ones_row[:], 1.0)

            # ---- persistent activations ----
            qT = [apool.tile([128, S], bf16, tag=f"qT{m}", name=f"qT{m}")
                  for m in range(CG // 128)]
            kTt = [apool.tile([128, S], bf16, tag=f"kT{m}", name=f"kT{m}")
                   for m in range(CG // 128)]
            vst = [apool.tile([128, HPG, DH + 1], bf16, tag=f"v{m}",
                              name=f"v{m}")
                   for m in range(S // 128)]
            aoT = [apool.tile([128, S], bf16, tag=f"aoT{m}", name=f"aoT{m}")
                   for m in range(CG // 128)]

            for st in range(S // 128):
                nc.gpsimd.memset(vst[st][:, :, DH:DH + 1], 1.0)

            # ---- work units ----
            def qk_unit(which, mi, n):
                # one psum-group of the q/k projection: out tile
                # qT/kT[mi][:, n*512:(n+1)*512]; bias added during the
                # PSUM->SBUF copy (q on ACT, k on DVE to split the load).
                wsb, out, bcol = ((wq_sb, qT, mi) if which == "q"
                                  else (wk_sb, kTt, 4 + mi))
                ps = sp.tile([128, 512], f32, tag="ps",
                             name=f"qk{which}{mi}_{n}")
                for k in range(KT):
                    nc.tensor.matmul(
                        ps[:],
                        wsb[:, mi * 1024 + k * 128:mi * 1024 + (k + 1) * 128],
                        xt[k][:, n * 512:(n + 1) * 512],
                        start=(k == 0), stop=(k == KT - 1))
                dst = out[mi][:, n * 512:(n + 1) * 512]
                if which == "q":
                    nc.scalar.activation(dst, ps[:], Ident,
                                         bias=bqk[:, bcol:bcol + 1])
                else:
                    nc.vector.tensor_scalar_add(dst, ps[:],
                                                bqk[:, bcol:bcol + 1])
                return KT

            def v_unit(p, st):
                # v rows [st] for head pair p (2 heads, 128 cols), natural
                # orientation; bias via K=1 ones x bv matmul; one strided
                # copy into the 65-col-per-head layout.
                ps = sp.tile([128, 2, DH], f32, tag="ps", name=f"v{p}_{st}")
                for k in range(KT):
                    nc.tensor.matmul(
                        ps[:], xt[k][:, st * 128:(st + 1) * 128],
                        wv_sb[:, k * 512 + p * 128:k * 512 + (p + 1) * 128],
                        start=(k == 0), stop=False)
                nc.tensor.matmul(ps[:], ones_row[:],
                                 bv_row[:, p * 128:(p + 1) * 128],
                                 start=False, stop=True)
                nc.vector.tensor_copy(vst[st][:, 2 * p:2 * p + 2, 0:DH],
                                      ps[:])
                return KT + 1

            def out_unit(st):
                # out-projection for 128 query rows: y[st] = aoT[:, st].T @ wo
                ys = ypool.tile([128, 1024], f32, tag="ys", name=f"ys{st}")
                for n in range(2):
                    ps = sp.tile([128, 512], f32, tag="ps",
                                 name=f"yps{st}_{n}")
                    for k in range(CG // 128):
                        nc.tensor.matmul(
                            ps[:],
                            aoT[k][:, st * 128:(st + 1) * 128],
                            wo_sb[:, k * 1024 + n * 512:k * 1024 + (n + 1) * 512],
                            start=(k == 0), stop=(k == CG // 128 - 1))
                    nc.vector.tensor_copy(ys[:, n * 512:(n + 1) * 512], ps[:])
                nc.sync.dma_start(y_d[st * 128:(st + 1) * 128, :], ys[:])
                return 2 * (CG // 128)

            # filler queue: (key, fn) drained between attention groups.
            # drain(key) force-issues everything up to and including key —
            # required when a later attention op depends on a unit's output:
            # the PE queue is in-order, so a dependency on a not-yet-issued
            # unit would deadlock (its matmuls would sit behind the stalled
            # consumer).
            fill = deque()
            issued = set()

            def pump(mm_budget):
                while mm_budget > 0 and fill:
                    k, fn = fill.popleft()
                    issued.add(k)
                    mm_budget -= fn()

            def drain(key):
                if key in issued:
                    return
                while fill:
                    k, fn = fill.popleft()
                    issued.add(k)
                    fn()
                    if k == key:
                        return

            # ---- upfront: qk m0 tiles needed by pair-0 ib1 scores ----
            # ib1 scores need qT[0][:, 1024:2048] (n=2,3) and kT[0] fully.
            for n in (2, 3):
                qk_unit("q", 0, n)
            for n in range(4):
                qk_unit("k", 0, n)

            # filler supply, in dependency order
            for st in range(S // 128):
                fill.append((f"v0_{st}", lambda st=st: v_unit(0, st)))
            for n in range(4):
                fill.append((f"qkq1_{n}", lambda n=n: qk_unit("q", 1, n)))
                fill.append((f"qkk1_{n}", lambda n=n: qk_unit("k", 1, n)))
            for st in range(S // 128):
                fill.append((f"v1_{st}", lambda st=st: v_unit(1, st)))
            for n in range(4):
                fill.append((f"qkq2_{n}", lambda n=n: qk_unit("q", 2, n)))
                fill.append((f"qkk2_{n}", lambda n=n: qk_unit("k", 2, n)))
            for st in range(S // 128):
                fill.append((f"v2_{st}", lambda st=st: v_unit(2, st)))
            for n in range(4):
                fill.append((f"qkq3_{n}", lambda n=n: qk_unit("q", 3, n)))
                fill.append((f"qkk3_{n}", lambda n=n: qk_unit("k", 3, n)))
            for st in range(S // 128):
                fill.append((f"v3_{st}", lambda st=st: v_unit(3, st)))
            for n in (0, 1):
                fill.append((f"qkq0_{n}", lambda n=n: qk_unit("q", 0, n)))

            # ---- attention ----
            def norm_sub(av, p, sub, ib):
                # normalize head (2p+sub), both 512-halves at once: the
                # denominator row (ones column of v) divides the AV psum
                # during the copy into aoT.  custom-DVE ops must not read
                # PSUM, so the denominator row is staged through SBUF.
                po = DH * sub
                dn = spool.tile([1, IB], f32, tag="den", name="dn")
                nc.vector.tensor_copy(dn[:], av[DH:DH + 1, :])
                rc = spool.tile([1, IB], f32, tag="recip", name="rc")
                nc.vector.reciprocal_approx_fast(rc[:], dn[:])
                rb = rpool.tile([DH, IB], f32, tag="rbcast", name="rb")
                nc.gpsimd.partition_broadcast(rb[:], rc[:])
                nc.vector.tensor_mul(
                    aoT[p][po:po + DH, ib * IB:(ib + 1) * IB],
                    av[0:DH, :], rb[:])

            def attn_pair(p, ib, pump_mm):
                njt = (ib + 1) * (IB // 128)
                dstart = njt - (IB // 128)
                last = [dstart + 3, njt - 1]
                avs = [avp.tile([DH + 1, IB], f32, tag="av",
                                name=f"av{p}_{ib}_{s}") for s in range(2)]
                pend = [deque(), deque()]
                for jt in range(njt + SKEW):
                    if jt < njt:
                        off = jt - dstart
                        c0 = 128 * off if off > 0 else 0
                        scps = []
                        for sub in range(2):
                            po = DH * sub
                            ps = sp.tile([128, IB], f32, tag="ps",
                                         name=f"ps{p}_{ib}_{jt}_{sub}")
                            for lo, hi in _halves(c0):
                                nc.tensor.matmul(
                                    ps[:, lo:hi],
                                    kTt[p][po:po + DH,
                                           jt * 128:(jt + 1) * 128],
                                    qT[p][po:po + DH,
                                          ib * IB + lo:ib * IB + hi],
                                    start=True, stop=True)
                            scps.append(ps)
                        for sub in range(2):
                            et = epool.tile([128, IB], bf16, tag="expT",
                                            name="et")
                            nc.scalar.activation(
                                et[:, c0:IB], scps[sub][:, c0:IB],
                                Exp, scale=float(DH) ** -0.5)
                            if jt >= dstart:
                                nc.vector.tensor_mul(
                                    et[:, c0:c0 + 128],
                                    et[:, c0:c0 + 128], tri[:])
                            pend[sub].append((jt, et, c0))
                    if jt % 2 == 1 or jt >= njt:
                        pump(pump_mm)
                        for sub in range(2):
                            h = 2 * p + sub
                            while pend[sub] and (
                                    len(pend[sub]) > SKEW or jt >= njt):
                                jt0, et, c0 = pend[sub].popleft()
                                drain(f"v{p}_{jt0}")
                                for lo, hi in _halves(c0):
                                    nc.tensor.matmul(
                                        avs[sub][:, lo:hi],
                                        vst[jt0][:, h, :],
                                        et[:, lo:hi],
                                        start=(jt0 == 0),
                                        stop=(jt0 == last[hi // 512 - 1]))
                                if jt0 == njt - 1:
                                    norm_sub(avs[sub], p, sub, ib)

            SKEW = 2
            # ib=1 first: its out-projection rows become ib=0-phase filler
            for p in range(HPG // 2):
                attn_pair(p, 1, pump_mm=10)
                if p < 3:
                    drain(f"qkk{p + 1}_3")
            drain("qkq0_1")
            for st in range(8, 16):
                fill.append((f"o{st}", lambda st=st: out_unit(st)))
            for p in range(HPG // 2):
                attn_pair(p, 0, pump_mm=5)
            while fill:
                k, fn = fill.popleft()
                issued.add(k)
                fn()
            for st in range(0, 8):
                out_unit(st)

    nc.compile()
    return nc


def _halves(c0):
    # the two 512-wide PSUM-bank column ranges, narrowed to the causally
    # valid region [c0, IB)
    for n in range(IB // 512):
        lo, hi = max(n * 512, c0), (n + 1) * 512
        if lo < hi:
            yield lo, hi


def _shard_inputs(x, w_qkv, b_qkv, w_out):
    # keep key j (partition) <= query i (free column): upper triangle
    tri = np.triu(np.ones((128, 128))).astype(BF16)
    in_maps = []
    for c in range(N_CORES):
        b, g = c // G, c % G
        sl = slice(g * CG, (g + 1) * CG)
        wq = w_qkv[:, 0 * D:1 * D][:, sl].astype(BF16)   # [1024, 512]
        wk = w_qkv[:, 1 * D:2 * D][:, sl].astype(BF16)
        wv = w_qkv[:, 2 * D:3 * D][:, sl].astype(BF16)
        wo = w_out[sl, :].astype(BF16)                   # [512, 1024]
        # m-major pack [128, 4096]: cols m*1024 + k*128
        wq_p = np.concatenate(
            [np.concatenate([wq[k * 128:(k + 1) * 128,
                                m * 128:(m + 1) * 128] for k in range(KT)],
                            axis=1) for m in range(4)], axis=1)
        wk_p = np.concatenate(
            [np.concatenate([wk[k * 128:(k + 1) * 128,
                                m * 128:(m + 1) * 128] for k in range(KT)],
                            axis=1) for m in range(4)], axis=1)
        # k-major pack [128, 4096]: cols k*512
        wv_p = np.concatenate([wv[k * 128:(k + 1) * 128, :]
                               for k in range(KT)], axis=1)
        wo_p = np.concatenate([wo[k * 128:(k + 1) * 128, :]
                               for k in range(4)], axis=1)
        bq = b_qkv[0 * D:1 * D][sl].reshape(4, 128).T    # [128, 4]
        bk = b_qkv[1 * D:2 * D][sl].reshape(4, 128).T
        in_maps.append({
            "xT": np.ascontiguousarray(x[b].T).astype(BF16),
            "wq": np.ascontiguousarray(wq_p).astype(BF16),
            "wk": np.ascontiguousarray(wk_p).astype(BF16),
            "wv": np.ascontiguousarray(wv_p).astype(BF16),
            "wo": np.ascontiguousarray(wo_p).astype(BF16),
            "bqk": np.ascontiguousarray(
                np.concatenate([bq, bk], axis=1)).astype(np.float32),
            "bv": b_qkv[2 * D:3 * D][sl].reshape(1, CG).astype(BF16),
            "tri": tri,
        })
    return in_maps


def kernel(x, w_qkv, b_qkv, w_out, b_out):
    from concourse.bass_utils import run_bass_kernel_spmd

    x = np.asarray(x, np.float32)
    w_qkv = np.asarray(w_qkv, np.float32)
    b_qkv = np.asarray(b_qkv, np.float32)
    w_out = np.asarray(w_out, np.float32)
    b_out = np.asarray(b_out, np.float32)

    if "nc" not in _cache:
        _cache["nc"] = _build_program()
    nc = _cache["nc"]

    in_maps = _shard_inputs(x, w_qkv, b_qkv, w_out)
    res = run_bass_kernel_spmd(nc, in_maps, core_ids=list(range(N_CORES)))
    _cache["last_result"] = res

    y = np.empty((B, S, D), np.float32)
    for b in range(B):
        y[b] = res.results[G * b]["y"] + res.results[G * b + 1]["y"] + b_out
    return y


# revision 11
# speedup vs baseline: 1.1163x; 1.1163x over previous
"""Trainium2 Bass kernel for causal multi-head attention (v2).

Problem: B=4, S=2048, D=1024, H=16 heads (d_head=64), fp32 I/O.
    qkv = x @ w_qkv + b_qkv ; causal softmax attention ; out @ w_out + b_out

Sharding over 8 NeuronCores: data-parallel over batch (4) x
tensor-parallel over head-groups (2 groups of 8 heads). Core c handles
batch c//2, head-group c%2. No collectives: each core returns its
partial out-projection; the host sums the two group partials per batch
and adds b_out.

v2 schedule (vs v1): the kernel is ACT(exp)-heavy in attention and
PE-heavy in the projections; v1 ran them serially, so the PE idled
waiting on exp in attention, HAM re-throttled it to half clock, and
the whole attention phase crawled.  v2 keeps one dense in-order PE
stream: attention starts as soon as head-pair 0's q/k m-tile is
projected, and all remaining projection work (qk m-tiles 1-3, v per
pair, out-projection) is held in a filler queue that is drained
between scores/AV groups.  The PE never waits on the ACT: every
dependency point has independent matmul work queued behind it.
ib=1 (queries 1024-2047) is processed first so the out-projection of
those rows becomes filler for the lighter ib=0 phase.

Layout notes: qT/kT = w.T @ xT (transposed acts, no on-device
transposes), v natural [s, h, 65] with a ones column per head so the
AV matmul also yields the softmax denominator; scoresT[j,i] per head
with K=64 — the two heads of an m-tile sit on partitions 0:64/64:128
so their scores matmuls land on different PE row-groups and run
concurrently (2x); max-free softmax on ScalarE (|logit| < ~7); causal
handled by narrowing to [c0, IB) plus one 128x128 triangular mask on
diagonal blocks; weights arrive as host-packed single-DMA tiles.
"""

import sys

if "/opt/trn_rl_repo" not in sys.path:
    sys.path.insert(0, "/opt/trn_rl_repo")

from collections import deque

import numpy as np
import ml_dtypes

B, S, D = 4, 2048, 1024
H, DH = 16, 64
G = 2                # tensor-parallel head groups
HPG = H // G         # heads per group (8)
CG = HPG * DH        # channel cols per group (512)
N_CORES = 8
BF16 = ml_dtypes.bfloat16

KT = D // 128        # 8 contraction k-tiles for the projections
IB = 1024            # i-block (query positions per attention block)
NIB = S // IB        # 2

_cache = {}


def _build_program():
    import concourse.tile as tile
    from concourse import bacc, mybir

    f32 = mybir.dt.float32
    bf16 = mybir.dt.bfloat16
    Exp = mybir.ActivationFunctionType.Exp
    Ident = mybir.ActivationFunctionType.Identity

    nc = bacc.Bacc("TRN2", target_bir_lowering=False, debug=False,
                   num_devices=N_CORES)

    xT_d = nc.dram_tensor("xT", [D, S], bf16, kind="ExternalInput").ap()
    # m-major pack: cols m*1024 + k*128 hold w[k-tile rows, m-tile cols]
    wq_d = nc.dram_tensor("wq", [128, 4096], bf16, kind="ExternalInput").ap()
    wk_d = nc.dram_tensor("wk", [128, 4096], bf16, kind="ExternalInput").ap()
    # k-major pack: cols k*512 hold w[k-tile rows, :]
    wv_d = nc.dram_tensor("wv", [128, 4096], bf16, kind="ExternalInput").ap()
    wo_d = nc.dram_tensor("wo", [128, 4096], bf16, kind="ExternalInput").ap()
    bqk_d = nc.dram_tensor("bqk", [128, 8], f32, kind="ExternalInput").ap()
    bv_d = nc.dram_tensor("bv", [1, CG], bf16, kind="ExternalInput").ap()
    tri_d = nc.dram_tensor("tri", [128, 128], bf16, kind="ExternalInput").ap()
    y_d = nc.dram_tensor("y", [S, D], f32, kind="ExternalOutput").ap()

    with tile.TileContext(nc) as tc:
        with (
            tc.tile_pool(name="consts", bufs=1) as cpool,
            tc.tile_pool(name="acts", bufs=1) as apool,
            tc.tile_pool(name="exps", bufs=12) as epool,
            tc.tile_pool(name="small", bufs=4) as spool,
            tc.tile_pool(name="rbc", bufs=2) as rpool,
            tc.tile_pool(name="ystage", bufs=3) as ypool,
            tc.tile_pool(name="psum_s", bufs=2, space="PSUM") as sp,
            tc.tile_pool(name="psum_av", bufs=2, space="PSUM") as avp,
        ):
            # ---- constants: few large DMAs, gating tiles first ----
            wq_sb = cpool.tile([128, 4096], bf16, tag="wq", name="wq_sb")
            wk_sb = cpool.tile([128, 4096], bf16, tag="wk", name="wk_sb")
            wv_sb = cpool.tile([128, 4096], bf16, tag="wv", name="wv_sb")
            wo_sb = cpool.tile([128, 4096], bf16, tag="wo", name="wo_sb")
            bqk = cpool.tile([128, 8], f32, tag="bqk", name="bqk")
            bv_row = cpool.tile([1, CG], bf16, tag="bv", name="bv_row")
            tri = cpool.tile([128, 128], bf16, tag="tri", name="tri")
            xt = [cpool.tile([128, S], bf16, tag=f"xt{k}", name=f"xt{k}")
                  for k in range(KT)]

            nc.sync.dma_start(wq_sb[:, 0:1024], wq_d[:, 0:1024])
            nc.sync.dma_start(xt[0][:], xT_d[0:128, :])
            nc.sync.dma_start(wk_sb[:, 0:1024], wk_d[:, 0:1024])
            nc.sync.dma_start(bqk[:], bqk_d[:])
            nc.sync.dma_start(bv_row[:], bv_d[:])
            nc.sync.dma_start(tri[:], tri_d[:])
            nc.sync.dma_start(xt[1][:], xT_d[128:256, :])
            nc.sync.dma_start(wv_sb[:], wv_d[:])
            for k in range(2, KT):
                nc.sync.dma_start(xt[k][:], xT_d[k * 128:(k + 1) * 128, :])
            nc.sync.dma_start(wq_sb[:, 1024:4096], wq_d[:, 1024:4096])
            nc.sync.dma_start(wk_sb[:, 1024:4096], wk_d[:, 1024:4096])
            nc.sync.dma_start(wo_sb[:], wo_d[:])

            ones_row = cpool.tile([1, 128], bf16, tag="ones", name="ones_row")
            nc.gpsimd.memset(ones_row[:], 1.0)

            # ---- persistent activations ----
            qT = [apool.tile([128, S], bf16, tag=f"qT{m}", name=f"qT{m}")
                  for m in range(CG // 128)]
            kTt = [apool.tile([128, S], bf16, tag=f"kT{m}", name=f"kT{m}")
                   for m in range(CG // 128)]
            vst = [apool.tile([128, HPG, DH + 1], bf16, tag=f"v{m}",
                              name=f"v{m}")
                   for m in range(S // 128)]
            aoT = [apool.tile([128, S], bf16, tag=f"aoT{m}", name=f"aoT{m}")
                   for m in range(CG // 128)]

            for st in range(S // 128):
                nc.gpsimd.memset(vst[st][:, :, DH:DH + 1], 1.0)

            # ---- work units ----
            def qk_unit(which, mi, n):
                # one psum-group of the q/k projection: out tile
                # qT/kT[mi][:, n*512:(n+1)*512]; bias added during the
                # PSUM->SBUF copy (q on ACT, k on DVE to split the load).
                wsb, out, bcol = ((wq_sb, qT, mi) if which == "q"
                                  else (wk_sb, kTt, 4 + mi))
                ps = sp.tile([128, 512], f32, tag="ps",
                             name=f"qk{which}{mi}_{n}")
                for k in range(KT):
                    nc.tensor.matmul(
                        ps[:],
                        wsb[:, mi * 1024 + k * 128:mi * 1024 + (k + 1) * 128],
                        xt[k][:, n * 512:(n + 1) * 512],
                        start=(k == 0), stop=(k == KT - 1))
                dst = out[mi][:, n * 512:(n + 1) * 512]
                if which == "q":
                    nc.scalar.activation(dst, ps[:], Ident,
                                         bias=bqk[:, bcol:bcol + 1])
                else:
                    nc.vector.tensor_scalar_add(dst, ps[:],
                                                bqk[:, bcol:bcol + 1])
                return KT

            def v_unit(st):
                # v rows [st], all 8 heads (N=512 keeps the PE streaming
                # ahead of LDWEIGHTS); bias via K=1 ones x bv matmul; one
                # strided copy into the 65-col-per-head layout.
                ps = sp.tile([128, HPG, DH], f32, tag="ps", name=f"v{st}")
                for k in range(KT):
                    nc.tensor.matmul(
                        ps[:], xt[k][:, st * 128:(st + 1) * 128],
                        wv_sb[:, k * 512:(k + 1) * 512],
                        start=(k == 0), stop=False)
                nc.tensor.matmul(ps[:], ones_row[:], bv_row[:],
                                 start=False, stop=True)
                nc.vector.tensor_copy(vst[st][:, :, 0:DH], ps[:])
                return KT + 1

            def out_unit(st):
                # out-projection for 128 query rows: y[st] = aoT[:, st].T @ wo
                ys = ypool.tile([128, 1024], f32, tag="ys", name=f"ys{st}")
                for n in range(2):
                    ps = sp.tile([128, 512], f32, tag="ps",
                                 name=f"yps{st}_{n}")
                    for k in range(CG // 128):
                        nc.tensor.matmul(
                            ps[:],
                            aoT[k][:, st * 128:(st + 1) * 128],
                            wo_sb[:, k * 1024 + n * 512:k * 1024 + (n + 1) * 512],
                            start=(k == 0), stop=(k == CG // 128 - 1))
                    nc.vector.tensor_copy(ys[:, n * 512:(n + 1) * 512], ps[:])
                nc.sync.dma_start(y_d[st * 128:(st + 1) * 128, :], ys[:])
                return 2 * (CG // 128)

            # filler queue: (key, fn) drained between attention groups.
            # drain(key) force-issues everything up to and including key —
            # required when a later attention op depends on a unit's output:
            # the PE queue is in-order, so a dependency on a not-yet-issued
            # unit would deadlock (its matmuls would sit behind the stalled
            # consumer).
            fill = deque()
            issued = set()

            def pump(mm_budget):
                while mm_budget > 0 and fill:
                    k, fn = fill.popleft()
                    issued.add(k)
                    mm_budget -= fn()

            def drain(key):
                if key in issued:
                    return
                while fill:
                    k, fn = fill.popleft()
                    issued.add(k)
                    fn()
                    if k == key:
                        return

            def qk_key(which, mi, n):
                return f"qk{which}{mi}_{n}"

            def add_qk(which, mi, n):
                fill.append((qk_key(which, mi, n),
                             lambda: qk_unit(which, mi, n)))

            # ---- upfront: the m0 n0/n1 halves gate pair-0 ib0 scores ----
            for n in (0, 1):
                qk_unit("q", 0, n)
                qk_unit("k", 0, n)

            # ib0-phase filler: v rows 0-7 (drained just-in-time by the
            # AVs) interleaved with the n0/n1 projection halves of pairs
            # 1-3 (force-drained before each pair's scores).
            qk01 = [(w, mi, n) for mi in (1, 2, 3) for n in (0, 1)
                    for w in ("q", "k")]
            for st in range(8):
                fill.append((f"v_{st}", lambda st=st: v_unit(st)))
                for u in qk01[2 * st:2 * st + 2]:
                    add_qk(*u)

            # ---- attention ----
            def norm_sub(av, p, sub, ib):
                # normalize head (2p+sub), both 512-halves at once: the
                # denominator row (ones column of v) divides the AV psum
                # during the copy into aoT.  custom-DVE ops must not read
                # PSUM, so the denominator row is staged through SBUF.
                po = DH * sub
                dn = spool.tile([1, IB], f32, tag="den", name="dn")
                nc.vector.tensor_copy(dn[:], av[DH:DH + 1, :])
                rc = spool.tile([1, IB], f32, tag="recip", name="rc")
                nc.vector.reciprocal_approx_fast(rc[:], dn[:])
                rb = rpool.tile([DH, IB], f32, tag="rbcast", name="rb")
                nc.gpsimd.partition_broadcast(rb[:], rc[:])
                nc.vector.tensor_mul(
                    aoT[p][po:po + DH, ib * IB:(ib + 1) * IB],
                    av[0:DH, :], rb[:])

            def attn_pair(p, ib, pump_mm):
                njt = (ib + 1) * (IB // 128)
                dstart = njt - (IB // 128)
                last = [dstart + 3, njt - 1]
                avs = [avp.tile([DH + 1, IB], f32, tag="av",
                                name=f"av{p}_{ib}_{s}") for s in range(2)]
                pend = [deque(), deque()]
                for jt in range(njt + SKEW):
                    if jt < njt:
                        off = jt - dstart
                        c0 = 128 * off if off > 0 else 0
                        scps = []
                        for sub in range(2):
                            po = DH * sub
                            ps = sp.tile([128, IB], f32, tag="ps",
                                         name=f"ps{p}_{ib}_{jt}_{sub}")
                            for lo, hi in _halves(c0):
                                nc.tensor.matmul(
                                    ps[:, lo:hi],
                                    kTt[p][po:po + DH,
                                           jt * 128:(jt + 1) * 128],
                                    qT[p][po:po + DH,
                                          ib * IB + lo:ib * IB + hi],
                                    start=True, stop=True)
                            scps.append(ps)
                        for sub in range(2):
                            et = epool.tile([128, IB], bf16, tag="expT",
                                            name="et")
                            nc.scalar.activation(
                                et[:, c0:IB], scps[sub][:, c0:IB],
                                Exp, scale=float(DH) ** -0.5)
                            if jt >= dstart:
                                nc.vector.tensor_mul(
                                    et[:, c0:c0 + 128],
                                    et[:, c0:c0 + 128], tri[:])
                            pend[sub].append((jt, et, c0))
                    if jt == 8 and ib == 1:
                        # j-tiles 8-15 read the n2/n3 half of kT
                        drain(qk_key("k", p, 2))
                        drain(qk_key("k", p, 3))
                    if jt % 2 == 1 or jt >= njt:
                        pump(pump_mm)
                        for sub in range(2):
                            h = 2 * p + sub
                            while pend[sub] and (
                                    len(pend[sub]) > SKEW or jt >= njt):
                                jt0, et, c0 = pend[sub].popleft()
                                drain(f"v_{jt0}")
                                for lo, hi in _halves(c0):
                                    nc.tensor.matmul(
                                        avs[sub][:, lo:hi],
                                        vst[jt0][:, h, :],
                                        et[:, lo:hi],
                                        start=(jt0 == 0),
                                        stop=(jt0 == last[hi // 512 - 1]))
                                if jt0 == njt - 1:
                                    norm_sub(avs[sub], p, sub, ib)

            SKEW = 2
            # ---- ib0 phase: scores/AV on queries 0-1023 over v rows 0-7,
            # with v and the pairs' n0/n1 projections as PE filler ----
            for p in range(HPG // 2):
                if p > 0:
                    drain(qk_key("q", p, 1))
                    drain(qk_key("k", p, 1))
                attn_pair(p, 0, pump_mm=5)

            # ib1-phase filler: remaining projection halves (n2/n3,
            # JIT-drained per pair), v rows 8-15 (JIT by the AVs), and the
            # out-projection of the now-finished ib0 query rows.
            qk23 = [(w, mi, n) for mi in range(4) for n in (2, 3)
                    for w in ("q", "k")]
            for st in range(8, 16):
                fill.append((f"v_{st}", lambda st=st: v_unit(st)))
                for u in qk23[2 * (st - 8):2 * (st - 8) + 2]:
                    add_qk(*u)
            for st in range(0, 8):
                fill.append((f"o{st}", lambda st=st: out_unit(st)))

            for p in range(HPG // 2):
                drain(qk_key("q", p, 2))
                drain(qk_key("q", p, 3))
                attn_pair(p, 1, pump_mm=6)
            while fill:
                k, fn = fill.popleft()
                issued.add(k)
                fn()
            for st in range(8, 16):
                out_unit(st)

    nc.compile()
    return nc


def _halves(c0):
    # the two 512-wide PSUM-bank column ranges, narrowed to the causally
    # valid region [c0, IB)
    for n in range(IB // 512):
        lo, hi = max(n * 512, c0), (n + 1) * 512
        if lo < hi:
            yield lo, hi


def _shard_inputs(x, w_qkv, b_qkv, w_out):
    # keep key j (partition) <= query i (free column): upper triangle
    tri = np.triu(np.ones((128, 128))).astype(BF16)
    in_maps = []
    for c in range(N_CORES):
        b, g = c // G, c % G
        sl = slice(g * CG, (g + 1) * CG)
        wq = w_qkv[:, 0 * D:1 * D][:, sl].astype(BF16)   # [1024, 512]
        wk = w_qkv[:, 1 * D:2 * D][:, sl].astype(BF16)
        wv = w_qkv[:, 2 * D:3 * D][:, sl].astype(BF16)
        wo = w_out[sl, :].astype(BF16)                   # [512, 1024]
        # m-major pack [128, 4096]: cols m*1024 + k*128
        wq_p = np.concatenate(
            [np.concatenate([wq[k * 128:(k + 1) * 128,
                                m * 128:(m + 1) * 128] for k in range(KT)],
                            axis=1) for m in range(4)], axis=1)
        wk_p = np.concatenate(
            [np.concatenate([wk[k * 128:(k + 1) * 128,
                                m * 128:(m + 1) * 128] for k in range(KT)],
                            axis=1) for m in range(4)], axis=1)
        # k-major pack [128, 4096]: cols k*512
        wv_p = np.concatenate([wv[k * 128:(k + 1) * 128, :]
                               for k in range(KT)], axis=1)
        wo_p = np.concatenate([wo[k * 128:(k + 1) * 128, :]
                               for k in range(4)], axis=1)
        bq = b_qkv[0 * D:1 * D][sl].reshape(4, 128).T    # [128, 4]
        bk = b_qkv[1 * D:2 * D][sl].reshape(4, 128).T
        in_maps.append({
            "xT": np.ascontiguousarray(x[b].T).astype(BF16),
            "wq": np.ascontiguousarray(wq_p).astype(BF16),
            "wk": np.ascontiguousarray(wk_p).astype(BF16),
            "wv": np.ascontiguousarray(wv_p).astype(BF16),
            "wo": np.ascontiguousarray(wo_p).astype(BF16),
            "bqk": np.ascontiguousarray(
                np.concatenate([bq, bk], axis=1)).astype(np.float32),
            "bv": b_qkv[2 * D:3 * D][sl].reshape(1, CG).astype(BF16),
            "tri": tri,
        })
    return in_maps


def kernel(x, w_qkv, b_qkv, w_out, b_out):
    from concourse.bass_utils import run_bass_kernel_spmd

    x = np.asarray(x, np.float32)
    w_qkv = np.asarray(w_qkv, np.float32)
    b_qkv = np.asarray(b_qkv, np.float32)
    w_out = np.asarray(w_out, np.float32)
    b_out = np.asarray(b_out, np.float32)

    if "nc" not in _cache:
        _cache["nc"] = _build_program()
    nc = _cache["nc"]

    in_maps = _shard_inputs(x, w_qkv, b_qkv, w_out)
    res = run_bass_kernel_spmd(nc, in_maps, core_ids=list(range(N_CORES)))
    _cache["last_result"] = res

    y = np.empty((B, S, D), np.float32)
    for b in range(B):
        y[b] = res.results[G * b]["y"] + res.results[G * b + 1]["y"] + b_out
    return y


# revision 18
# speedup vs baseline: 1.1236x; 1.0065x over previous
"""Trainium2 Bass kernel for causal multi-head attention (v2).

Problem: B=4, S=2048, D=1024, H=16 heads (d_head=64), fp32 I/O.
    qkv = x @ w_qkv + b_qkv ; causal softmax attention ; out @ w_out + b_out

Sharding over 8 NeuronCores: data-parallel over batch (4) x
tensor-parallel over head-groups (2 groups of 8 heads). Core c handles
batch c//2, head-group c%2. No collectives: each core returns its
partial out-projection; the host sums the two group partials per batch
and adds b_out.

v2 schedule (vs v1): the kernel is ACT(exp)-heavy in attention and
PE-heavy in the projections; v1 ran them serially, so the PE idled
waiting on exp in attention, HAM re-throttled it to half clock, and
the whole attention phase crawled.  v2 keeps one dense in-order PE
stream: attention starts as soon as head-pair 0's q/k m-tile is
projected, and all remaining projection work (qk m-tiles 1-3, v per
pair, out-projection) is held in a filler queue that is drained
between scores/AV groups.  The PE never waits on the ACT: every
dependency point has independent matmul work queued behind it.
ib=1 (queries 1024-2047) is processed first so the out-projection of
those rows becomes filler for the lighter ib=0 phase.

Layout notes: qT/kT = w.T @ xT (transposed acts, no on-device
transposes), v natural [s, h, 65] with a ones column per head so the
AV matmul also yields the softmax denominator; scoresT[j,i] per head
with K=64 — the two heads of an m-tile sit on partitions 0:64/64:128
so their scores matmuls land on different PE row-groups and run
concurrently (2x); max-free softmax on ScalarE (|logit| < ~7); causal
handled by narrowing to [c0, IB) plus one 128x128 triangular mask on
diagonal blocks; weights arrive as host-packed single-DMA tiles.
"""

import sys

if "/opt/trn_rl_repo" not in sys.path:
    sys.path.insert(0, "/opt/trn_rl_repo")

from collections import deque

import numpy as np
import ml_dtypes

B, S, D = 4, 2048, 1024
H, DH = 16, 64
G = 2                # tensor-parallel head groups
HPG = H // G         # heads per group (8)
CG = HPG * DH        # channel cols per group (512)
N_CORES = 8
BF16 = ml_dtypes.bfloat16

KT = D // 128        # 8 contraction k-tiles for the projections
IB = 1024            # i-block (query positions per attention block)
NIB = S // IB        # 2

_cache = {}


def _build_program():
    import concourse.tile as tile
    from concourse import bacc, mybir

    f32 = mybir.dt.float32
    bf16 = mybir.dt.bfloat16
    Exp = mybir.ActivationFunctionType.Exp
    Ident = mybir.ActivationFunctionType.Identity

    nc = bacc.Bacc("TRN2", target_bir_lowering=False, debug=False,
                   num_devices=N_CORES)

    xT_d = nc.dram_tensor("xT", [D, S], bf16, kind="ExternalInput").ap()
    # m-major pack: cols m*1024 + k*128 hold w[k-tile rows, m-tile cols]
    wq_d = nc.dram_tensor("wq", [128, 4096], bf16, kind="ExternalInput").ap()
    wk_d = nc.dram_tensor("wk", [128, 4096], bf16, kind="ExternalInput").ap()
    # k-major pack: cols k*512 hold w[k-tile rows, :]
    wv_d = nc.dram_tensor("wv", [128, 4096], bf16, kind="ExternalInput").ap()
    wo_d = nc.dram_tensor("wo", [128, 4096], bf16, kind="ExternalInput").ap()
    bqk_d = nc.dram_tensor("bqk", [128, 8], f32, kind="ExternalInput").ap()
    bv_d = nc.dram_tensor("bv", [1, CG], bf16, kind="ExternalInput").ap()
    tri_d = nc.dram_tensor("tri", [128, 128], bf16, kind="ExternalInput").ap()
    y_d = nc.dram_tensor("y", [S, D], f32, kind="ExternalOutput").ap()

    with tile.TileContext(nc) as tc:
        with (
            tc.tile_pool(name="consts", bufs=1) as cpool,
            tc.tile_pool(name="acts", bufs=1) as apool,
            tc.tile_pool(name="exps", bufs=12) as epool,
            tc.tile_pool(name="small", bufs=4) as spool,
            tc.tile_pool(name="rbc", bufs=2) as rpool,
            tc.tile_pool(name="ystage", bufs=3) as ypool,
            tc.tile_pool(name="psum_s", bufs=2, space="PSUM") as sp,
            tc.tile_pool(name="psum_av", bufs=2, space="PSUM") as avp,
        ):
            # ---- constants: few large DMAs, gating tiles first ----
            wq_sb = cpool.tile([128, 4096], bf16, tag="wq", name="wq_sb")
            wk_sb = cpool.tile([128, 4096], bf16, tag="wk", name="wk_sb")
            wv_sb = cpool.tile([128, 4096], bf16, tag="wv", name="wv_sb")
            wo_sb = cpool.tile([128, 4096], bf16, tag="wo", name="wo_sb")
            bqk = cpool.tile([128, 8], f32, tag="bqk", name="bqk")
            bv_row = cpool.tile([1, CG], bf16, tag="bv", name="bv_row")
            tri = cpool.tile([128, 128], bf16, tag="tri", name="tri")
            xt = [cpool.tile([128, S], bf16, tag=f"xt{k}", name=f"xt{k}")
                  for k in range(KT)]

            nc.sync.dma_start(wq_sb[:, 0:1024], wq_d[:, 0:1024])
            nc.sync.dma_start(xt[0][:], xT_d[0:128, :])
            nc.sync.dma_start(wk_sb[:, 0:1024], wk_d[:, 0:1024])
            nc.sync.dma_start(bqk[:], bqk_d[:])
            nc.sync.dma_start(bv_row[:], bv_d[:])
            nc.sync.dma_start(tri[:], tri_d[:])
            nc.sync.dma_start(xt[1][:], xT_d[128:256, :])
            nc.sync.dma_start(wv_sb[:], wv_d[:])
            for k in range(2, KT):
                nc.sync.dma_start(xt[k][:], xT_d[k * 128:(k + 1) * 128, :])
            nc.sync.dma_start(wq_sb[:, 1024:4096], wq_d[:, 1024:4096])
            nc.sync.dma_start(wk_sb[:, 1024:4096], wk_d[:, 1024:4096])
            nc.sync.dma_start(wo_sb[:], wo_d[:])

            ones_row = cpool.tile([1, 128], bf16, tag="ones", name="ones_row")
            nc.gpsimd.memset(ones_row[:], 1.0)

            # ---- persistent activations ----
            qT = [apool.tile([128, S], bf16, tag=f"qT{m}", name=f"qT{m}")
                  for m in range(CG // 128)]
            kTt = [apool.tile([128, S], bf16, tag=f"kT{m}", name=f"kT{m}")
                   for m in range(CG // 128)]
            vst = [apool.tile([128, HPG, DH + 1], bf16, tag=f"v{m}",
                              name=f"v{m}")
                   for m in range(S // 128)]
            aoT = [apool.tile([128, S], bf16, tag=f"aoT{m}", name=f"aoT{m}")
                   for m in range(CG // 128)]

            for st in range(S // 128):
                nc.gpsimd.memset(vst[st][:, :, DH:DH + 1], 1.0)

            # ---- work units ----
            def qk_unit(which, mi, n):
                # one psum-group of the q/k projection: out tile
                # qT/kT[mi][:, n*512:(n+1)*512]; bias added during the
                # PSUM->SBUF copy (q on ACT, k on DVE to split the load).
                wsb, out, bcol = ((wq_sb, qT, mi) if which == "q"
                                  else (wk_sb, kTt, 4 + mi))
                ps = sp.tile([128, 512], f32, tag="ps",
                             name=f"qk{which}{mi}_{n}")
                for k in range(KT):
                    nc.tensor.matmul(
                        ps[:],
                        wsb[:, mi * 1024 + k * 128:mi * 1024 + (k + 1) * 128],
                        xt[k][:, n * 512:(n + 1) * 512],
                        start=(k == 0), stop=(k == KT - 1))
                dst = out[mi][:, n * 512:(n + 1) * 512]
                if which == "q":
                    nc.scalar.activation(dst, ps[:], Ident,
                                         bias=bqk[:, bcol:bcol + 1])
                    led["act"] += 512 * ACT_NS + ACT_FIX
                else:
                    nc.vector.tensor_scalar_add(dst, ps[:],
                                                bqk[:, bcol:bcol + 1])
                led["pe"] += KT * 512 * PE_NS
                return KT

            def v_unit(st):
                # v rows [st], all 8 heads (N=512 keeps the PE streaming
                # ahead of LDWEIGHTS); bias via K=1 ones x bv matmul; one
                # strided copy into the 65-col-per-head layout.
                ps = sp.tile([128, HPG, DH], f32, tag="ps", name=f"v{st}")
                for k in range(KT):
                    nc.tensor.matmul(
                        ps[:], xt[k][:, st * 128:(st + 1) * 128],
                        wv_sb[:, k * 512:(k + 1) * 512],
                        start=(k == 0), stop=False)
                nc.tensor.matmul(ps[:], ones_row[:], bv_row[:],
                                 start=False, stop=True)
                nc.vector.tensor_copy(vst[st][:, :, 0:DH], ps[:])
                led["pe"] += (KT + 1) * 512 * PE_NS
                return KT + 1

            def out_unit(st):
                # out-projection for 128 query rows: y[st] = aoT[:, st].T @ wo
                ys = ypool.tile([128, 1024], f32, tag="ys", name=f"ys{st}")
                for n in range(2):
                    ps = sp.tile([128, 512], f32, tag="ps",
                                 name=f"yps{st}_{n}")
                    for k in range(CG // 128):
                        nc.tensor.matmul(
                            ps[:],
                            aoT[k][:, st * 128:(st + 1) * 128],
                            wo_sb[:, k * 1024 + n * 512:k * 1024 + (n + 1) * 512],
                            start=(k == 0), stop=(k == CG // 128 - 1))
                    nc.vector.tensor_copy(ys[:, n * 512:(n + 1) * 512], ps[:])
                nc.sync.dma_start(y_d[st * 128:(st + 1) * 128, :], ys[:])
                led["pe"] += 2 * (CG // 128) * 512 * PE_NS
                return 2 * (CG // 128)

            # filler queue: (key, fn) drained between attention groups.
            # pump() issues filler only while the cumulative PE work issued
            # trails the cumulative ACT work issued plus a lead margin —
            # this spreads the (fixed) filler supply exactly against the
            # exp-heavy stretches so neither engine starves.
            # drain(key) force-issues everything up to and including key —
            # required when a later attention op depends on a unit's output:
            # the PE queue is in-order, so a dependency on a not-yet-issued
            # unit would deadlock (its matmuls would sit behind the stalled
            # consumer).
            fill = deque()
            issued = set()
            led = {"pe": 0.0, "act": 0.0}
            LEAD_NS = 4000.0
            PE_NS = 1.0 / 2.4          # ns per output column, warm
            ACT_NS = 1.0 / 1.2         # ns per column
            ACT_FIX = 190.0            # per-instruction access/init cost

            def pump():
                while fill and led["pe"] < led["act"] + LEAD_NS:
                    k, fn = fill.popleft()
                    issued.add(k)
                    fn()

            def drain(key):
                if key in issued:
                    return
                while fill:
                    k, fn = fill.popleft()
                    issued.add(k)
                    fn()
                    if k == key:
                        return

            def qk_key(which, mi, n):
                return f"qk{which}{mi}_{n}"

            def add_qk(which, mi, n):
                fill.append((qk_key(which, mi, n),
                             lambda: qk_unit(which, mi, n)))

            # ---- upfront: the m0 n0/n1 halves gate pair-0 ib0 scores ----
            for n in (0, 1):
                qk_unit("q", 0, n)
                qk_unit("k", 0, n)

            # ib0-phase filler: v rows 0-7 (drained just-in-time by the
            # AVs) interleaved with the n0/n1 projection halves of pairs
            # 1-3 (force-drained before each pair's scores).
            qk01 = [(w, mi, n) for mi in (1, 2, 3) for n in (0, 1)
                    for w in ("q", "k")]
            for st in range(8):
                fill.append((f"v_{st}", lambda st=st: v_unit(st)))
                for u in qk01[2 * st:2 * st + 2]:
                    add_qk(*u)

            # ---- attention ----
            def norm_sub(av, p, sub, ib):
                # normalize head (2p+sub), both 512-halves at once: the
                # denominator row (ones column of v) divides the AV psum
                # during the copy into aoT.  custom-DVE ops must not read
                # PSUM, so the denominator row is staged through SBUF.
                po = DH * sub
                dn = spool.tile([1, IB], f32, tag="den", name="dn")
                nc.vector.tensor_copy(dn[:], av[DH:DH + 1, :])
                rc = spool.tile([1, IB], f32, tag="recip", name="rc")
                nc.vector.reciprocal_approx_fast(rc[:], dn[:])
                rb = rpool.tile([DH, IB], f32, tag="rbcast", name="rb")
                nc.gpsimd.partition_broadcast(rb[:], rc[:])
                nc.vector.tensor_mul(
                    aoT[p][po:po + DH, ib * IB:(ib + 1) * IB],
                    av[0:DH, :], rb[:])

            def attn_pair(p, ib):
                njt = (ib + 1) * (IB // 128)
                dstart = njt - (IB // 128)
                last = [dstart + 3, njt - 1]
                avs = [avp.tile([DH + 1, IB], f32, tag="av",
                                name=f"av{p}_{ib}_{s}") for s in range(2)]
                pend = [deque(), deque()]
                for jt in range(njt + SKEW):
                    if jt < njt:
                        off = jt - dstart
                        c0 = 128 * off if off > 0 else 0
                        scps = []
                        for sub in range(2):
                            po = DH * sub
                            ps = sp.tile([128, IB], f32, tag="ps",
                                         name=f"ps{p}_{ib}_{jt}_{sub}")
                            for lo, hi in _halves(c0):
                                nc.tensor.matmul(
                                    ps[:, lo:hi],
                                    kTt[p][po:po + DH,
                                           jt * 128:(jt + 1) * 128],
                                    qT[p][po:po + DH,
                                          ib * IB + lo:ib * IB + hi],
                                    start=True, stop=True)
                            scps.append(ps)
                        # the two subs' K=64 scores run on different PE
                        # row-groups concurrently: charge one sub's columns
                        led["pe"] += (IB - c0) * PE_NS
                        for sub in range(2):
                            et = epool.tile([128, IB], bf16, tag="expT",
                                            name="et")
                            nc.scalar.activation(
                                et[:, c0:IB], scps[sub][:, c0:IB],
                                Exp, scale=float(DH) ** -0.5)
                            led["act"] += (IB - c0) * ACT_NS + ACT_FIX
                            if jt >= dstart:
                                nc.vector.tensor_mul(
                                    et[:, c0:c0 + 128],
                                    et[:, c0:c0 + 128], tri[:])
                            pend[sub].append((jt, et, c0))
                    if jt == 8 and ib == 1:
                        # j-tiles 8-15 read the n2/n3 half of kT
                        drain(qk_key("k", p, 2))
                        drain(qk_key("k", p, 3))
                    if jt % 2 == 1 or jt >= njt:
                        pump()
                        for sub in range(2):
                            h = 2 * p + sub
                            while pend[sub] and (
                                    len(pend[sub]) > SKEW or jt >= njt):
                                jt0, et, c0 = pend[sub].popleft()
                                drain(f"v_{jt0}")
                                for lo, hi in _halves(c0):
                                    nc.tensor.matmul(
                                        avs[sub][:, lo:hi],
                                        vst[jt0][:, h, :],
                                        et[:, lo:hi],
                                        start=(jt0 == 0),
                                        stop=(jt0 == last[hi // 512 - 1]))
                                led["pe"] += (IB - c0) * PE_NS
                                if jt0 == njt - 1:
                                    norm_sub(avs[sub], p, sub, ib)

            SKEW = 2
            # ---- ib0 phase: scores/AV on queries 0-1023 over v rows 0-7,
            # with v and the pairs' n0/n1 projections as PE filler ----
            for p in range(HPG // 2):
                if p > 0:
                    drain(qk_key("q", p, 1))
                    drain(qk_key("k", p, 1))
                attn_pair(p, 0)

            # ib1-phase filler: remaining projection halves (n2/n3,
            # JIT-drained per pair), v rows 8-15 (JIT by the AVs), and the
            # out-projection of the now-finished ib0 query rows.
            qk23 = [(w, mi, n) for mi in range(4) for n in (2, 3)
                    for w in ("q", "k")]
            for st in range(8, 16):
                fill.append((f"v_{st}", lambda st=st: v_unit(st)))
                for u in qk23[2 * (st - 8):2 * (st - 8) + 2]:
                    add_qk(*u)
            for st in range(0, 8):
                fill.append((f"o{st}", lambda st=st: out_unit(st)))

            for p in range(HPG // 2):
                drain(qk_key("q", p, 2))
                drain(qk_key("q", p, 3))
                attn_pair(p, 1)
            while fill:
                k, fn = fill.popleft()
                issued.add(k)
                fn()
            for st in range(8, 16):
                out_unit(st)

    nc.compile()
    return nc


def _halves(c0):
    # the two 512-wide PSUM-bank column ranges, narrowed to the causally
    # valid region [c0, IB)
    for n in range(IB // 512):
        lo, hi = max(n * 512, c0), (n + 1) * 512
        if lo < hi:
            yield lo, hi


def _shard_inputs(x, w_qkv, b_qkv, w_out):
    # keep key j (partition) <= query i (free column): upper triangle
    tri = np.triu(np.ones((128, 128))).astype(BF16)
    in_maps = []
    for c in range(N_CORES):
        b, g = c // G, c % G
        sl = slice(g * CG, (g + 1) * CG)
        wq = w_qkv[:, 0 * D:1 * D][:, sl].astype(BF16)   # [1024, 512]
        wk = w_qkv[:, 1 * D:2 * D][:, sl].astype(BF16)
        wv = w_qkv[:, 2 * D:3 * D][:, sl].astype(BF16)
        wo = w_out[sl, :].astype(BF16)                   # [512, 1024]
        # m-major pack [128, 4096]: cols m*1024 + k*128
        wq_p = np.concatenate(
            [np.concatenate([wq[k * 128:(k + 1) * 128,
                                m * 128:(m + 1) * 128] for k in range(KT)],
                            axis=1) for m in range(4)], axis=1)
        wk_p = np.concatenate(
            [np.concatenate([wk[k * 128:(k + 1) * 128,
                                m * 128:(m + 1) * 128] for k in range(KT)],
                            axis=1) for m in range(4)], axis=1)
        # k-major pack [128, 4096]: cols k*512
        wv_p = np.concatenate([wv[k * 128:(k + 1) * 128, :]
                               for k in range(KT)], axis=1)
        wo_p = np.concatenate([wo[k * 128:(k + 1) * 128, :]
                               for k in range(4)], axis=1)
        bq = b_qkv[0 * D:1 * D][sl].reshape(4, 128).T    # [128, 4]
        bk = b_qkv[1 * D:2 * D][sl].reshape(4, 128).T
        in_maps.append({
            "xT": np.ascontiguousarray(x[b].T).astype(BF16),
            "wq": np.ascontiguousarray(wq_p).astype(BF16),
            "wk": np.ascontiguousarray(wk_p).astype(BF16),
            "wv": np.ascontiguousarray(wv_p).astype(BF16),
            "wo": np.ascontiguousarray(wo_p).astype(BF16),
            "bqk": np.ascontiguousarray(
                np.concatenate([bq, bk], axis=1)).astype(np.float32),
            "bv": b_qkv[2 * D:3 * D][sl].reshape(1, CG).astype(BF16),
            "tri": tri,
        })
    return in_maps


def kernel(x, w_qkv, b_qkv, w_out, b_out):
    from concourse.bass_utils import run_bass_kernel_spmd

    x = np.asarray(x, np.float32)
    w_qkv = np.asarray(w_qkv, np.float32)
    b_qkv = np.asarray(b_qkv, np.float32)
    w_out = np.asarray(w_out, np.float32)
    b_out = np.asarray(b_out, np.float32)

    if "nc" not in _cache:
        _cache["nc"] = _build_program()
    nc = _cache["nc"]

    in_maps = _shard_inputs(x, w_qkv, b_qkv, w_out)
    res = run_bass_kernel_spmd(nc, in_maps, core_ids=list(range(N_CORES)))
    _cache["last_result"] = res

    y = np.empty((B, S, D), np.float32)
    for b in range(B):
        y[b] = res.results[G * b]["y"] + res.results[G * b + 1]["y"] + b_out
    return y


# revision 24
# speedup vs baseline: 1.1988x; 1.0670x over previous
"""Trainium2 Bass kernel for causal multi-head attention (v2).

Problem: B=4, S=2048, D=1024, H=16 heads (d_head=64), fp32 I/O.
    qkv = x @ w_qkv + b_qkv ; causal softmax attention ; out @ w_out + b_out

Sharding over 8 NeuronCores: data-parallel over batch (4) x
tensor-parallel over head-groups (2 groups of 8 heads). Core c handles
batch c//2, head-group c%2. No collectives: each core returns its
partial out-projection; the host sums the two group partials per batch
and adds b_out.

v2 schedule (vs v1): the kernel is ACT(exp)-heavy in attention and
PE-heavy in the projections; v1 ran them serially, so the PE idled
waiting on exp in attention, HAM re-throttled it to half clock, and
the whole attention phase crawled.  v2 keeps one dense in-order PE
stream: attention starts as soon as head-pair 0's q/k m-tile is
projected, and all remaining projection work (qk m-tiles 1-3, v per
pair, out-projection) is held in a filler queue that is drained
between scores/AV groups.  The PE never waits on the ACT: every
dependency point has independent matmul work queued behind it.
ib=1 (queries 1024-2047) is processed first so the out-projection of
those rows becomes filler for the lighter ib=0 phase.

Layout notes: qT/kT = w.T @ xT (transposed acts, no on-device
transposes), v natural [s, h, 65] with a ones column per head so the
AV matmul also yields the softmax denominator; scoresT[j,i] per head
with K=64 — the two heads of an m-tile sit on partitions 0:64/64:128
so their scores matmuls land on different PE row-groups and run
concurrently (2x); max-free softmax on ScalarE (|logit| < ~7); causal
handled by narrowing to [c0, IB) plus one 128x128 triangular mask on
diagonal blocks; weights arrive as host-packed single-DMA tiles.
"""

import sys

if "/opt/trn_rl_repo" not in sys.path:
    sys.path.insert(0, "/opt/trn_rl_repo")

from collections import deque

import numpy as np
import ml_dtypes

B, S, D = 4, 2048, 1024
H, DH = 16, 64
G = 2                # tensor-parallel head groups
HPG = H // G         # heads per group (8)
CG = HPG * DH        # channel cols per group (512)
N_CORES = 8
BF16 = ml_dtypes.bfloat16

KT = D // 128        # 8 contraction k-tiles for the projections
IB = 1024            # i-block (query positions per attention block)
NIB = S // IB        # 2

_cache = {}


def _build_program():
    import concourse.tile as tile
    from concourse import bacc, mybir

    f32 = mybir.dt.float32
    bf16 = mybir.dt.bfloat16
    Exp = mybir.ActivationFunctionType.Exp
    Ident = mybir.ActivationFunctionType.Identity

    nc = bacc.Bacc("TRN2", target_bir_lowering=False, debug=False,
                   num_devices=N_CORES)

    xT_d = nc.dram_tensor("xT", [D, S], bf16, kind="ExternalInput").ap()
    # m-major pack: cols m*1024 + k*128 hold w[k-tile rows, m-tile cols]
    wq_d = nc.dram_tensor("wq", [128, 4096], bf16, kind="ExternalInput").ap()
    wk_d = nc.dram_tensor("wk", [128, 4096], bf16, kind="ExternalInput").ap()
    # k-major pack: cols k*512 hold w[k-tile rows, :]
    wv_d = nc.dram_tensor("wv", [128, 4096], bf16, kind="ExternalInput").ap()
    wo_d = nc.dram_tensor("wo", [128, 4096], bf16, kind="ExternalInput").ap()
    bqk_d = nc.dram_tensor("bqk", [128, 8], f32, kind="ExternalInput").ap()
    bv_d = nc.dram_tensor("bv", [1, CG], bf16, kind="ExternalInput").ap()
    tri_d = nc.dram_tensor("tri", [128, 128], bf16, kind="ExternalInput").ap()
    y_d = nc.dram_tensor("y", [S, D], f32, kind="ExternalOutput").ap()

    with tile.TileContext(nc) as tc:
        with (
            tc.tile_pool(name="consts", bufs=1) as cpool,
            tc.tile_pool(name="acts", bufs=1) as apool,
            tc.tile_pool(name="exps", bufs=12) as epool,
            tc.tile_pool(name="small", bufs=4) as spool,
            tc.tile_pool(name="rbc", bufs=2) as rpool,
            tc.tile_pool(name="ystage", bufs=3) as ypool,
            tc.tile_pool(name="psum_s", bufs=2, space="PSUM") as sp,
            tc.tile_pool(name="psum_av", bufs=2, space="PSUM") as avp,
        ):
            # ---- constants: few large DMAs, gating tiles first ----
            wq_sb = cpool.tile([128, 4096], bf16, tag="wq", name="wq_sb")
            wk_sb = cpool.tile([128, 4096], bf16, tag="wk", name="wk_sb")
            wv_sb = cpool.tile([128, 4096], bf16, tag="wv", name="wv_sb")
            wo_sb = cpool.tile([128, 4096], bf16, tag="wo", name="wo_sb")
            bqk = cpool.tile([128, 8], f32, tag="bqk", name="bqk")
            bv_row = cpool.tile([1, CG], bf16, tag="bv", name="bv_row")
            tri = cpool.tile([128, 128], bf16, tag="tri", name="tri")
            xt = [cpool.tile([128, S], bf16, tag=f"xt{k}", name=f"xt{k}")
                  for k in range(KT)]

            nc.sync.dma_start(wq_sb[:, 0:1024], wq_d[:, 0:1024])
            nc.sync.dma_start(xt[0][:], xT_d[0:128, :])
            nc.sync.dma_start(wk_sb[:, 0:1024], wk_d[:, 0:1024])
            nc.sync.dma_start(bqk[:], bqk_d[:])
            nc.sync.dma_start(bv_row[:], bv_d[:])
            nc.sync.dma_start(tri[:], tri_d[:])
            nc.sync.dma_start(xt[1][:], xT_d[128:256, :])
            nc.sync.dma_start(wv_sb[:], wv_d[:])
            for k in range(2, KT):
                nc.sync.dma_start(xt[k][:], xT_d[k * 128:(k + 1) * 128, :])
            nc.sync.dma_start(wq_sb[:, 1024:4096], wq_d[:, 1024:4096])
            nc.sync.dma_start(wk_sb[:, 1024:4096], wk_d[:, 1024:4096])
            nc.sync.dma_start(wo_sb[:], wo_d[:])

            ones_row = cpool.tile([1, 128], bf16, tag="ones", name="ones_row")
            nc.gpsimd.memset(ones_row[:], 1.0)

            # ---- persistent activations ----
            qT = [apool.tile([128, S], bf16, tag=f"qT{m}", name=f"qT{m}")
                  for m in range(CG // 128)]
            kTt = [apool.tile([128, S], bf16, tag=f"kT{m}", name=f"kT{m}")
                   for m in range(CG // 128)]
            vst = [apool.tile([128, HPG, DH + 1], bf16, tag=f"v{m}",
                              name=f"v{m}")
                   for m in range(S // 128)]
            aoT = [apool.tile([128, S], bf16, tag=f"aoT{m}", name=f"aoT{m}")
                   for m in range(CG // 128)]

            for st in range(S // 128):
                nc.gpsimd.memset(vst[st][:, :, DH:DH + 1], 1.0)

            # ---- work units ----
            def qk_unit(which, mi, n):
                # one psum-group of the q/k projection: out tile
                # qT/kT[mi][:, n*512:(n+1)*512]; bias added during the
                # PSUM->SBUF copy (q on ACT, k on DVE to split the load).
                wsb, out, bcol = ((wq_sb, qT, mi) if which == "q"
                                  else (wk_sb, kTt, 4 + mi))
                ps = sp.tile([128, 512], f32, tag="ps",
                             name=f"qk{which}{mi}_{n}")
                for k in range(KT):
                    nc.tensor.matmul(
                        ps[:],
                        wsb[:, mi * 1024 + k * 128:mi * 1024 + (k + 1) * 128],
                        xt[k][:, n * 512:(n + 1) * 512],
                        start=(k == 0), stop=(k == KT - 1))
                # both biases on DVE: the ACT queue stays pure exp so the
                # scores->exp->psum-free chain never queues behind copies
                dst = out[mi][:, n * 512:(n + 1) * 512]
                nc.vector.tensor_scalar_add(dst, ps[:],
                                            bqk[:, bcol:bcol + 1])
                return KT

            def v_unit(st):
                # v rows [st], all 8 heads (N=512 keeps the PE streaming
                # ahead of LDWEIGHTS); bias via K=1 ones x bv matmul; one
                # strided copy into the 65-col-per-head layout.
                ps = sp.tile([128, HPG, DH], f32, tag="ps", name=f"v{st}")
                for k in range(KT):
                    nc.tensor.matmul(
                        ps[:], xt[k][:, st * 128:(st + 1) * 128],
                        wv_sb[:, k * 512:(k + 1) * 512],
                        start=(k == 0), stop=False)
                nc.tensor.matmul(ps[:], ones_row[:], bv_row[:],
                                 start=False, stop=True)
                nc.vector.tensor_copy(vst[st][:, :, 0:DH], ps[:])
                return KT + 1

            def out_unit(st):
                # out-projection for 128 query rows: y[st] = aoT[:, st].T @ wo
                ys = ypool.tile([128, 1024], f32, tag="ys", name=f"ys{st}")
                for n in range(2):
                    ps = sp.tile([128, 512], f32, tag="ps",
                                 name=f"yps{st}_{n}")
                    for k in range(CG // 128):
                        nc.tensor.matmul(
                            ps[:],
                            aoT[k][:, st * 128:(st + 1) * 128],
                            wo_sb[:, k * 1024 + n * 512:k * 1024 + (n + 1) * 512],
                            start=(k == 0), stop=(k == CG // 128 - 1))
                    nc.vector.tensor_copy(ys[:, n * 512:(n + 1) * 512], ps[:])
                nc.sync.dma_start(y_d[st * 128:(st + 1) * 128, :], ys[:])
                return 2 * (CG // 128)

            # filler queue: (key, fn) in ration order — each attention pair
            # is budgeted (allowance) the units the NEXT pair needs, so
            # filler spreads across the pair's j-tile groups instead of
            # bunching into a dense run at a pair boundary (bunching starves
            # the PE of inter-group work and lets HAM re-throttle it).
            # drain(key) force-issues everything up to and including key —
            # required when a later attention op depends on a unit's output:
            # the PE queue is in-order, so a dependency on a not-yet-issued
            # unit would deadlock (its matmuls would sit behind the stalled
            # consumer).
            fill = deque()
            issued = set()

            def pump(n_units):
                while n_units > 0 and fill:
                    k, fn = fill.popleft()
                    issued.add(k)
                    fn()
                    n_units -= 1

            def drain(key):
                if key in issued:
                    return
                while fill:
                    k, fn = fill.popleft()
                    issued.add(k)
                    fn()
                    if k == key:
                        return

            def qk_key(which, mi, n):
                return f"qk{which}{mi}_{n}"

            def add_qk(which, mi, n):
                fill.append((qk_key(which, mi, n),
                             lambda: qk_unit(which, mi, n)))

            # ---- upfront: the m0 n0/n1 halves gate pair-0 ib0 scores ----
            for n in (0, 1):
                qk_unit("q", 0, n)
                qk_unit("k", 0, n)

            # ib0-phase filler: v rows 0-7 (drained just-in-time by the
            # AVs) interleaved with the n0/n1 projection halves of pairs
            # 1-3 (force-drained before each pair's scores).
            qk01 = [(w, mi, n) for mi in (1, 2, 3) for n in (0, 1)
                    for w in ("q", "k")]
            for st in range(8):
                fill.append((f"v_{st}", lambda st=st: v_unit(st)))
                for u in qk01[2 * st:2 * st + 2]:
                    add_qk(*u)

            # ---- attention ----
            def norm_half(av, p, sub, ib, hf):
                # normalize one 512-col half of head (2p+sub): the
                # denominator row (ones column of v) divides the AV psum
                # during the copy into aoT.  Per-half so the early half's
                # psum bank frees mid-pair — the next pair's AV allocations
                # then never wait on a just-issued norm chain.  custom-DVE
                # ops must not read PSUM: the denominator goes via SBUF.
                po = DH * sub
                base = ib * IB + 512 * hf
                dn = spool.tile([1, 512], f32, tag="den", name="dn")
                nc.vector.tensor_copy(dn[:], av[DH:DH + 1, :])
                rc = spool.tile([1, 512], f32, tag="recip", name="rc")
                nc.vector.reciprocal_approx_fast(rc[:], dn[:])
                rb = rpool.tile([DH, 512], f32, tag="rbcast", name="rb")
                nc.gpsimd.partition_broadcast(rb[:], rc[:])
                nc.vector.tensor_mul(
                    aoT[p][po:po + DH, base:base + 512],
                    av[0:DH, :], rb[:])

            def attn_pair(p, ib, allowance):
                njt = (ib + 1) * (IB // 128)
                dstart = njt - (IB // 128)
                last = [dstart + 3, njt - 1]
                # av accumulators per (sub, half): 1 psum bank each,
                # released at their own stop+norm (half 0 mid-pair)
                avs = [[avp.tile([DH + 1, 512], f32, tag="av", bufs=4,
                                 name=f"av{p}_{ib}_{s}_{hf}")
                        for hf in range(2)] for s in range(2)]
                pend = [deque(), deque()]
                budget = [allowance]
                for jt in range(njt + SKEW):
                    if jt < njt:
                        off = jt - dstart
                        c0 = 128 * off if off > 0 else 0
                        scps = []
                        for sub in range(2):
                            po = DH * sub
                            ps = sp.tile([128, IB], f32, tag="ps",
                                         name=f"ps{p}_{ib}_{jt}_{sub}")
                            for lo, hi in _halves(c0):
                                nc.tensor.matmul(
                                    ps[:, lo:hi],
                                    kTt[p][po:po + DH,
                                           jt * 128:(jt + 1) * 128],
                                    qT[p][po:po + DH,
                                          ib * IB + lo:ib * IB + hi],
                                    start=True, stop=True)
                            scps.append(ps)
                        for sub in range(2):
                            et = epool.tile([128, IB], bf16, tag="expT",
                                            name="et")
                            nc.scalar.activation(
                                et[:, c0:IB], scps[sub][:, c0:IB],
                                Exp, scale=float(DH) ** -0.5)
                            if jt >= dstart:
                                nc.vector.tensor_mul(
                                    et[:, c0:c0 + 128],
                                    et[:, c0:c0 + 128], tri[:])
                            pend[sub].append((jt, et, c0))
                    if jt == 8 and ib == 1:
                        # j-tiles 8-15 read the n2/n3 half of kT
                        drain(qk_key("k", p, 2))
                        drain(qk_key("k", p, 3))
                    if jt % 2 == 1 or jt >= njt:
                        if budget[0] > 0:
                            pump(2)
                            budget[0] -= 2
                        for sub in range(2):
                            h = 2 * p + sub
                            while pend[sub] and (
                                    len(pend[sub]) > SKEW or jt >= njt):
                                jt0, et, c0 = pend[sub].popleft()
                                drain(f"v_{jt0}")
                                for lo, hi in _halves(c0):
                                    hf = hi // 512 - 1
                                    nc.tensor.matmul(
                                        avs[sub][hf][:, lo - 512 * hf:
                                                     hi - 512 * hf],
                                        vst[jt0][:, h, :],
                                        et[:, lo:hi],
                                        start=(jt0 == 0),
                                        stop=(jt0 == last[hf]))
                                    if jt0 == last[hf]:
                                        norm_half(avs[sub][hf], p, sub,
                                                  ib, hf)

            SKEW = 2
            # ---- ib0 phase: scores/AV on queries 0-1023 over v rows 0-7,
            # with v and the pairs' n0/n1 projections as PE filler ----
            for p in range(HPG // 2):
                if p > 0:
                    drain(qk_key("q", p, 1))
                    drain(qk_key("k", p, 1))
                attn_pair(p, 0, allowance=(12 if p == 0 else 4))

            # ib1-phase filler: remaining projection halves (n2/n3,
            # JIT-drained per pair), v rows 8-15 (JIT by the AVs), and the
            # out-projection of the now-finished ib0 query rows.
            qk23 = [(w, mi, n) for mi in range(4) for n in (2, 3)
                    for w in ("q", "k")]
            for st in range(8, 16):
                fill.append((f"v_{st}", lambda st=st: v_unit(st)))
                for u in qk23[2 * (st - 8):2 * (st - 8) + 2]:
                    add_qk(*u)
            for st in range(0, 8):
                fill.append((f"o{st}", lambda st=st: out_unit(st)))

            for p in range(HPG // 2):
                drain(qk_key("q", p, 2))
                drain(qk_key("q", p, 3))
                attn_pair(p, 1, allowance=(12 if p == 0 else 6))
            while fill:
                k, fn = fill.popleft()
                issued.add(k)
                fn()
            for st in range(8, 16):
                out_unit(st)

    nc.compile()
    return nc


def _halves(c0):
    # the two 512-wide PSUM-bank column ranges, narrowed to the causally
    # valid region [c0, IB)
    for n in range(IB // 512):
        lo, hi = max(n * 512, c0), (n + 1) * 512
        if lo < hi:
            yield lo, hi


def _shard_inputs(x, w_qkv, b_qkv, w_out):
    # keep key j (partition) <= query i (free column): upper triangle
    tri = np.triu(np.ones((128, 128))).astype(BF16)
    in_maps = []
    for c in range(N_CORES):
        b, g = c // G, c % G
        sl = slice(g * CG, (g + 1) * CG)
        wq = w_qkv[:, 0 * D:1 * D][:, sl].astype(BF16)   # [1024, 512]
        wk = w_qkv[:, 1 * D:2 * D][:, sl].astype(BF16)
        wv = w_qkv[:, 2 * D:3 * D][:, sl].astype(BF16)
        wo = w_out[sl, :].astype(BF16)                   # [512, 1024]
        # m-major pack [128, 4096]: cols m*1024 + k*128
        wq_p = np.concatenate(
            [np.concatenate([wq[k * 128:(k + 1) * 128,
                                m * 128:(m + 1) * 128] for k in range(KT)],
                            axis=1) for m in range(4)], axis=1)
        wk_p = np.concatenate(
            [np.concatenate([wk[k * 128:(k + 1) * 128,
                                m * 128:(m + 1) * 128] for k in range(KT)],
                            axis=1) for m in range(4)], axis=1)
        # k-major pack [128, 4096]: cols k*512
        wv_p = np.concatenate([wv[k * 128:(k + 1) * 128, :]
                               for k in range(KT)], axis=1)
        wo_p = np.concatenate([wo[k * 128:(k + 1) * 128, :]
                               for k in range(4)], axis=1)
        bq = b_qkv[0 * D:1 * D][sl].reshape(4, 128).T    # [128, 4]
        bk = b_qkv[1 * D:2 * D][sl].reshape(4, 128).T
        in_maps.append({
            "xT": np.ascontiguousarray(x[b].T).astype(BF16),
            "wq": np.ascontiguousarray(wq_p).astype(BF16),
            "wk": np.ascontiguousarray(wk_p).astype(BF16),
            "wv": np.ascontiguousarray(wv_p).astype(BF16),
            "wo": np.ascontiguousarray(wo_p).astype(BF16),
            "bqk": np.ascontiguousarray(
                np.concatenate([bq, bk], axis=1)).astype(np.float32),
            "bv": b_qkv[2 * D:3 * D][sl].reshape(1, CG).astype(BF16),
            "tri": tri,
        })
    return in_maps


def kernel(x, w_qkv, b_qkv, w_out, b_out):
    from concourse.bass_utils import run_bass_kernel_spmd

    x = np.asarray(x, np.float32)
    w_qkv = np.asarray(w_qkv, np.float32)
    b_qkv = np.asarray(b_qkv, np.float32)
    w_out = np.asarray(w_out, np.float32)
    b_out = np.asarray(b_out, np.float32)

    if "nc" not in _cache:
        _cache["nc"] = _build_program()
    nc = _cache["nc"]

    in_maps = _shard_inputs(x, w_qkv, b_qkv, w_out)
    res = run_bass_kernel_spmd(nc, in_maps, core_ids=list(range(N_CORES)))
    _cache["last_result"] = res

    y = np.empty((B, S, D), np.float32)
    for b in range(B):
        y[b] = res.results[G * b]["y"] + res.results[G * b + 1]["y"] + b_out
    return y


# revision 32
# speedup vs baseline: 1.2079x; 1.0076x over previous
"""Trainium2 Bass kernel for causal multi-head attention (v2).

Problem: B=4, S=2048, D=1024, H=16 heads (d_head=64), fp32 I/O.
    qkv = x @ w_qkv + b_qkv ; causal softmax attention ; out @ w_out + b_out

Sharding over 8 NeuronCores: data-parallel over batch (4) x
tensor-parallel over head-groups (2 groups of 8 heads). Core c handles
batch c//2, head-group c%2. No collectives: each core returns its
partial out-projection; the host sums the two group partials per batch
and adds b_out.

v2 schedule (vs v1): the kernel is ACT(exp)-heavy in attention and
PE-heavy in the projections; v1 ran them serially, so the PE idled
waiting on exp in attention, HAM re-throttled it to half clock, and
the whole attention phase crawled.  v2 keeps one dense in-order PE
stream: attention starts as soon as head-pair 0's q/k m-tile is
projected, and all remaining projection work (qk m-tiles 1-3, v per
pair, out-projection) is held in a filler queue that is drained
between scores/AV groups.  The PE never waits on the ACT: every
dependency point has independent matmul work queued behind it.
ib=1 (queries 1024-2047) is processed first so the out-projection of
those rows becomes filler for the lighter ib=0 phase.

Layout notes: qT/kT = w.T @ xT (transposed acts, no on-device
transposes), v natural [s, h, 65] with a ones column per head so the
AV matmul also yields the softmax denominator; scoresT[j,i] per head
with K=64 — the two heads of an m-tile sit on partitions 0:64/64:128
so their scores matmuls land on different PE row-groups and run
concurrently (2x); max-free softmax on ScalarE (|logit| < ~7); causal
handled by narrowing to [c0, IB) plus one 128x128 triangular mask on
diagonal blocks; weights arrive as host-packed single-DMA tiles.
"""

import sys

if "/opt/trn_rl_repo" not in sys.path:
    sys.path.insert(0, "/opt/trn_rl_repo")

from collections import deque

import numpy as np
import ml_dtypes

B, S, D = 4, 2048, 1024
H, DH = 16, 64
G = 2                # tensor-parallel head groups
HPG = H // G         # heads per group (8)
CG = HPG * DH        # channel cols per group (512)
N_CORES = 8
BF16 = ml_dtypes.bfloat16

KT = D // 128        # 8 contraction k-tiles for the projections
IB = 1024            # i-block (query positions per attention block)
NIB = S // IB        # 2

_cache = {}


def _build_program():
    import concourse.tile as tile
    from concourse import bacc, mybir

    f32 = mybir.dt.float32
    bf16 = mybir.dt.bfloat16
    Exp = mybir.ActivationFunctionType.Exp
    Ident = mybir.ActivationFunctionType.Identity

    nc = bacc.Bacc("TRN2", target_bir_lowering=False, debug=False,
                   num_devices=N_CORES)

    xT_d = nc.dram_tensor("xT", [D, S], bf16, kind="ExternalInput").ap()
    # m-major pack: cols m*1024 + k*128 hold w[k-tile rows, m-tile cols]
    wq_d = nc.dram_tensor("wq", [128, 4096], bf16, kind="ExternalInput").ap()
    wk_d = nc.dram_tensor("wk", [128, 4096], bf16, kind="ExternalInput").ap()
    # k-major pack: cols k*512 hold w[k-tile rows, :]
    wv_d = nc.dram_tensor("wv", [128, 4096], bf16, kind="ExternalInput").ap()
    wo_d = nc.dram_tensor("wo", [128, 4096], bf16, kind="ExternalInput").ap()
    bqk_d = nc.dram_tensor("bqk", [128, 8], f32, kind="ExternalInput").ap()
    bv_d = nc.dram_tensor("bv", [1, CG], bf16, kind="ExternalInput").ap()
    tri_d = nc.dram_tensor("tri", [128, 128], bf16, kind="ExternalInput").ap()
    y_d = nc.dram_tensor("y", [S, D], f32, kind="ExternalOutput").ap()

    with tile.TileContext(nc) as tc:
        with (
            tc.tile_pool(name="consts", bufs=1) as cpool,
            tc.tile_pool(name="acts", bufs=1) as apool,
            tc.tile_pool(name="exps", bufs=12) as epool,
            tc.tile_pool(name="small", bufs=4) as spool,
            tc.tile_pool(name="rbc", bufs=2) as rpool,
            tc.tile_pool(name="ystage", bufs=3) as ypool,
            tc.tile_pool(name="psum_s", bufs=2, space="PSUM") as sp,
            tc.tile_pool(name="psum_av", bufs=2, space="PSUM") as avp,
        ):
            # ---- constants: few large DMAs, gating tiles first ----
            wq_sb = cpool.tile([128, 4096], bf16, tag="wq", name="wq_sb")
            wk_sb = cpool.tile([128, 4096], bf16, tag="wk", name="wk_sb")
            wv_sb = cpool.tile([128, 4096], bf16, tag="wv", name="wv_sb")
            wo_sb = cpool.tile([128, 4096], bf16, tag="wo", name="wo_sb")
            bqk = cpool.tile([128, 8], f32, tag="bqk", name="bqk")
            bv_row = cpool.tile([1, CG], bf16, tag="bv", name="bv_row")
            tri = cpool.tile([128, 128], bf16, tag="tri", name="tri")
            xt = [cpool.tile([128, S], bf16, tag=f"xt{k}", name=f"xt{k}")
                  for k in range(KT)]

            nc.sync.dma_start(xt[0][:], xT_d[0:128, :])
            nc.sync.dma_start(wq_sb[:, 0:1024], wq_d[:, 0:1024])
            nc.sync.dma_start(wk_sb[:, 0:1024], wk_d[:, 0:1024])
            nc.sync.dma_start(xt[1][:], xT_d[128:256, :])
            nc.sync.dma_start(bqk[:], bqk_d[:])
            for k in range(2, KT):
                nc.sync.dma_start(xt[k][:], xT_d[k * 128:(k + 1) * 128, :])
            nc.sync.dma_start(tri[:], tri_d[:])
            nc.sync.dma_start(bv_row[:], bv_d[:])
            nc.sync.dma_start(wv_sb[:], wv_d[:])
            nc.sync.dma_start(wq_sb[:, 1024:4096], wq_d[:, 1024:4096])
            nc.sync.dma_start(wk_sb[:, 1024:4096], wk_d[:, 1024:4096])
            nc.sync.dma_start(wo_sb[:], wo_d[:])

            ones_row = cpool.tile([1, 128], bf16, tag="ones", name="ones_row")
            nc.gpsimd.memset(ones_row[:], 1.0)

            # ---- persistent activations ----
            qT = [apool.tile([128, S], bf16, tag=f"qT{m}", name=f"qT{m}")
                  for m in range(CG // 128)]
            kTt = [apool.tile([128, S], bf16, tag=f"kT{m}", name=f"kT{m}")
                   for m in range(CG // 128)]
            vst = [apool.tile([128, HPG, DH + 1], bf16, tag=f"v{m}",
                              name=f"v{m}")
                   for m in range(S // 128)]
            aoT = [apool.tile([128, S], bf16, tag=f"aoT{m}", name=f"aoT{m}")
                   for m in range(CG // 128)]

            for st in range(S // 128):
                nc.gpsimd.memset(vst[st][:, :, DH:DH + 1], 1.0)

            # ---- work units ----
            def qk_unit(which, mi, n):
                # one psum-group of the q/k projection: out tile
                # qT/kT[mi][:, n*512:(n+1)*512]; bias added during the
                # PSUM->SBUF copy (q on ACT, k on DVE to split the load).
                wsb, out, bcol = ((wq_sb, qT, mi) if which == "q"
                                  else (wk_sb, kTt, 4 + mi))
                ps = sp.tile([128, 512], f32, tag="ps",
                             name=f"qk{which}{mi}_{n}")
                for k in range(KT):
                    nc.tensor.matmul(
                        ps[:],
                        wsb[:, mi * 1024 + k * 128:mi * 1024 + (k + 1) * 128],
                        xt[k][:, n * 512:(n + 1) * 512],
                        start=(k == 0), stop=(k == KT - 1))
                # q-bias on ACT (idle in the projection-heavy stretches),
                # k-bias on DVE: splits the PSUM-evacuation load
                dst = out[mi][:, n * 512:(n + 1) * 512]
                if which == "q":
                    nc.scalar.activation(dst, ps[:], Ident,
                                         bias=bqk[:, bcol:bcol + 1])
                else:
                    nc.vector.tensor_scalar_add(dst, ps[:],
                                                bqk[:, bcol:bcol + 1])
                return KT

            def v_unit(st):
                # v rows [st], all 8 heads (N=512 keeps the PE streaming
                # ahead of LDWEIGHTS); bias via K=1 ones x bv matmul; one
                # strided copy into the 65-col-per-head layout.
                ps = sp.tile([128, HPG, DH], f32, tag="ps", name=f"v{st}")
                for k in range(KT):
                    nc.tensor.matmul(
                        ps[:], xt[k][:, st * 128:(st + 1) * 128],
                        wv_sb[:, k * 512:(k + 1) * 512],
                        start=(k == 0), stop=False)
                nc.tensor.matmul(ps[:], ones_row[:], bv_row[:],
                                 start=False, stop=True)
                nc.scalar.activation(vst[st][:, :, 0:DH], ps[:], Ident)
                return KT + 1

            def out_unit(st):
                # out-projection for 128 query rows: y[st] = aoT[:, st].T @ wo
                ys = ypool.tile([128, 1024], f32, tag="ys", name=f"ys{st}")
                for n in range(2):
                    ps = sp.tile([128, 512], f32, tag="ps",
                                 name=f"yps{st}_{n}")
                    for k in range(CG // 128):
                        nc.tensor.matmul(
                            ps[:],
                            aoT[k][:, st * 128:(st + 1) * 128],
                            wo_sb[:, k * 1024 + n * 512:k * 1024 + (n + 1) * 512],
                            start=(k == 0), stop=(k == CG // 128 - 1))
                    nc.vector.tensor_copy(ys[:, n * 512:(n + 1) * 512], ps[:])
                nc.sync.dma_start(y_d[st * 128:(st + 1) * 128, :], ys[:])
                return 2 * (CG // 128)

            # filler queue: (key, fn) in ration order — each attention pair
            # is budgeted (allowance) the units the NEXT pair needs, so
            # filler spreads across the pair's j-tile groups instead of
            # bunching into a dense run at a pair boundary (bunching starves
            # the PE of inter-group work and lets HAM re-throttle it).
            # drain(key) force-issues everything up to and including key —
            # required when a later attention op depends on a unit's output:
            # the PE queue is in-order, so a dependency on a not-yet-issued
            # unit would deadlock (its matmuls would sit behind the stalled
            # consumer).
            fill = deque()
            issued = set()

            def pump(n_units):
                while n_units > 0 and fill:
                    k, fn = fill.popleft()
                    issued.add(k)
                    fn()
                    n_units -= 1

            def drain(key):
                if key in issued:
                    return
                while fill:
                    k, fn = fill.popleft()
                    issued.add(k)
                    fn()
                    if k == key:
                        return

            def qk_key(which, mi, n):
                return f"qk{which}{mi}_{n}"

            def add_qk(which, mi, n):
                fill.append((qk_key(which, mi, n),
                             lambda: qk_unit(which, mi, n)))

            # ---- upfront: the m0 n0/n1 halves gate pair-0 ib0 scores ----
            for n in (0, 1):
                qk_unit("q", 0, n)
                qk_unit("k", 0, n)

            # ib0-phase filler: v rows 0-7 (drained just-in-time by the
            # AVs) interleaved with the n0/n1 projection halves of pairs
            # 1-3 (force-drained before each pair's scores).
            qk01 = [(w, mi, n) for mi in (1, 2, 3) for n in (0, 1)
                    for w in ("q", "k")]
            for st in range(8):
                fill.append((f"v_{st}", lambda st=st: v_unit(st)))
                for u in qk01[2 * st:2 * st + 2]:
                    add_qk(*u)

            # ---- attention ----
            def norm_half(av, p, sub, ib, hf):
                # normalize one 512-col half of head (2p+sub): the
                # denominator row (ones column of v) divides the AV psum
                # during the copy into aoT.  Per-half so the early half's
                # psum bank frees mid-pair — the next pair's AV allocations
                # then never wait on a just-issued norm chain.  custom-DVE
                # ops must not read PSUM: the denominator goes via SBUF.
                po = DH * sub
                base = ib * IB + 512 * hf
                dn = spool.tile([1, 512], f32, tag="den", name="dn")
                nc.vector.tensor_copy(dn[:], av[DH:DH + 1, :])
                rc = spool.tile([1, 512], f32, tag="recip", name="rc")
                nc.vector.reciprocal_approx_fast(rc[:], dn[:])
                rb = rpool.tile([DH, 512], f32, tag="rbcast", name="rb")
                nc.gpsimd.partition_broadcast(rb[:], rc[:])
                nc.vector.tensor_mul(
                    aoT[p][po:po + DH, base:base + 512],
                    av[0:DH, :], rb[:])

            def attn_pair(p, ib, allowance, on_group=None):
                njt = (ib + 1) * (IB // 128)
                dstart = njt - (IB // 128)
                last = [dstart + 3, njt - 1]
                # av accumulators per (sub, half): 1 psum bank each,
                # released at their own stop+norm (half 0 mid-pair)
                avs = [[avp.tile([DH + 1, 512], f32, tag="av", bufs=4,
                                 name=f"av{p}_{ib}_{s}_{hf}")
                        for hf in range(2)] for s in range(2)]
                pend = [deque(), deque()]
                budget = [allowance]
                for jt in range(njt + SKEW):
                    if jt < njt:
                        off = jt - dstart
                        c0 = 128 * off if off > 0 else 0
                        scps = []
                        for sub in range(2):
                            po = DH * sub
                            ps = sp.tile([128, IB], f32, tag="ps",
                                         name=f"ps{p}_{ib}_{jt}_{sub}")
                            for lo, hi in _halves(c0):
                                nc.tensor.matmul(
                                    ps[:, lo:hi],
                                    kTt[p][po:po + DH,
                                           jt * 128:(jt + 1) * 128],
                                    qT[p][po:po + DH,
                                          ib * IB + lo:ib * IB + hi],
                                    start=True, stop=True)
                            scps.append(ps)
                        for sub in range(2):
                            et = epool.tile([128, IB], bf16, tag="expT",
                                            name="et")
                            nc.scalar.activation(
                                et[:, c0:IB], scps[sub][:, c0:IB],
                                Exp, scale=float(DH) ** -0.5)
                            if jt >= dstart:
                                nc.vector.tensor_mul(
                                    et[:, c0:c0 + 128],
                                    et[:, c0:c0 + 128], tri[:])
                            pend[sub].append((jt, et, c0))
                    if jt == 8 and ib == 1:
                        # j-tiles 8-15 read the n2/n3 half of kT
                        drain(qk_key("k", p, 2))
                        drain(qk_key("k", p, 3))
                    if jt % 2 == 1 or jt >= njt:
                        if on_group is not None:
                            on_group(jt)
                        if budget[0] > 0:
                            pump(2)
                            budget[0] -= 2
                        for sub in range(2):
                            h = 2 * p + sub
                            while pend[sub] and (
                                    len(pend[sub]) > SKEW or jt >= njt):
                                jt0, et, c0 = pend[sub].popleft()
                                drain(f"v_{jt0}")
                                for lo, hi in _halves(c0):
                                    hf = hi // 512 - 1
                                    nc.tensor.matmul(
                                        avs[sub][hf][:, lo - 512 * hf:
                                                     hi - 512 * hf],
                                        vst[jt0][:, h, :],
                                        et[:, lo:hi],
                                        start=(jt0 == 0),
                                        stop=(jt0 == last[hf]))
                                    if jt0 == last[hf]:
                                        norm_half(avs[sub][hf], p, sub,
                                                  ib, hf)

            SKEW = 2
            # ---- ib0 phase: scores/AV on queries 0-1023 over v rows 0-7,
            # with v and the pairs' n0/n1 projections as PE filler ----
            for p in range(HPG // 2):
                if p > 0:
                    drain(qk_key("q", p, 1))
                    drain(qk_key("k", p, 1))
                attn_pair(p, 0, allowance=(12 if p == 0 else 6))

            # ib1-phase filler: remaining projection halves (n2/n3,
            # JIT-drained per pair), v rows 8-15 (JIT by the AVs), and the
            # out-projection of the now-finished ib0 query rows.
            qk23 = [(w, mi, n) for mi in range(4) for n in (2, 3)
                    for w in ("q", "k")]
            for st in range(8, 16):
                fill.append((f"v_{st}", lambda st=st: v_unit(st)))
                for u in qk23[2 * (st - 8):2 * (st - 8) + 2]:
                    add_qk(*u)
            for st in range(0, 8):
                fill.append((f"o{st}", lambda st=st: out_unit(st)))

            # out-projection rows 1024-1535 depend only on the ib1 half-0
            # norms (done by j-tile 11): release those units mid-pair-3 so
            # the tail shrinks to rows 1536-2047.
            o_early = {"done": False}

            def late_fill(jt):
                if jt >= 12 and not o_early["done"]:
                    o_early["done"] = True
                    for st in range(8, 12):
                        fill.append((f"o{st}", lambda st=st: out_unit(st)))

            for p in range(HPG // 2):
                drain(qk_key("q", p, 2))
                drain(qk_key("q", p, 3))
                attn_pair(p, 1, allowance=(12 if p == 0 else 7),
                          on_group=late_fill if p == 3 else None)
            while fill:
                k, fn = fill.popleft()
                issued.add(k)
                fn()
            for st in range(12, 16):
                out_unit(st)

    nc.compile()
    return nc


def _halves(c0):
    # the two 512-wide PSUM-bank column ranges, narrowed to the causally
    # valid region [c0, IB)
    for n in range(IB // 512):
        lo, hi = max(n * 512, c0), (n + 1) * 512
        if lo < hi:
            yield lo, hi


def _shard_inputs(x, w_qkv, b_qkv, w_out):
    # keep key j (partition) <= query i (free column): upper triangle
    tri = np.triu(np.ones((128, 128))).astype(BF16)
    in_maps = []
    for c in range(N_CORES):
        b, g = c // G, c % G
        sl = slice(g * CG, (g + 1) * CG)
        wq = w_qkv[:, 0 * D:1 * D][:, sl].astype(BF16)   # [1024, 512]
        wk = w_qkv[:, 1 * D:2 * D][:, sl].astype(BF16)
        wv = w_qkv[:, 2 * D:3 * D][:, sl].astype(BF16)
        wo = w_out[sl, :].astype(BF16)                   # [512, 1024]
        # m-major pack [128, 4096]: cols m*1024 + k*128
        wq_p = np.concatenate(
            [np.concatenate([wq[k * 128:(k + 1) * 128,
                                m * 128:(m + 1) * 128] for k in range(KT)],
                            axis=1) for m in range(4)], axis=1)
        wk_p = np.concatenate(
            [np.concatenate([wk[k * 128:(k + 1) * 128,
                                m * 128:(m + 1) * 128] for k in range(KT)],
                            axis=1) for m in range(4)], axis=1)
        # k-major pack [128, 4096]: cols k*512
        wv_p = np.concatenate([wv[k * 128:(k + 1) * 128, :]
                               for k in range(KT)], axis=1)
        wo_p = np.concatenate([wo[k * 128:(k + 1) * 128, :]
                               for k in range(4)], axis=1)
        bq = b_qkv[0 * D:1 * D][sl].reshape(4, 128).T    # [128, 4]
        bk = b_qkv[1 * D:2 * D][sl].reshape(4, 128).T
        in_maps.append({
            "xT": np.ascontiguousarray(x[b].T).astype(BF16),
            "wq": np.ascontiguousarray(wq_p).astype(BF16),
            "wk": np.ascontiguousarray(wk_p).astype(BF16),
            "wv": np.ascontiguousarray(wv_p).astype(BF16),
            "wo": np.ascontiguousarray(wo_p).astype(BF16),
            "bqk": np.ascontiguousarray(
                np.concatenate([bq, bk], axis=1)).astype(np.float32),
            "bv": b_qkv[2 * D:3 * D][sl].reshape(1, CG).astype(BF16),
            "tri": tri,
        })
    return in_maps


def kernel(x, w_qkv, b_qkv, w_out, b_out):
    from concourse.bass_utils import run_bass_kernel_spmd

    x = np.asarray(x, np.float32)
    w_qkv = np.asarray(w_qkv, np.float32)
    b_qkv = np.asarray(b_qkv, np.float32)
    w_out = np.asarray(w_out, np.float32)
    b_out = np.asarray(b_out, np.float32)

    if "nc" not in _cache:
        _cache["nc"] = _build_program()
    nc = _cache["nc"]

    in_maps = _shard_inputs(x, w_qkv, b_qkv, w_out)
    res = run_bass_kernel_spmd(nc, in_maps, core_ids=list(range(N_CORES)))
    _cache["last_result"] = res

    y = np.empty((B, S, D), np.float32)
    for b in range(B):
        y[b] = res.results[G * b]["y"] + res.results[G * b + 1]["y"] + b_out
    return y


# revision 33
# speedup vs baseline: 1.2583x; 1.0417x over previous
"""Trainium2 Bass kernel for causal multi-head attention (v2).

Problem: B=4, S=2048, D=1024, H=16 heads (d_head=64), fp32 I/O.
    qkv = x @ w_qkv + b_qkv ; causal softmax attention ; out @ w_out + b_out

Sharding over 8 NeuronCores: data-parallel over batch (4) x
tensor-parallel over head-groups (2 groups of 8 heads). Core c handles
batch c//2, head-group c%2. No collectives: each core returns its
partial out-projection; the host sums the two group partials per batch
and adds b_out.

v2 schedule (vs v1): the kernel is ACT(exp)-heavy in attention and
PE-heavy in the projections; v1 ran them serially, so the PE idled
waiting on exp in attention, HAM re-throttled it to half clock, and
the whole attention phase crawled.  v2 keeps one dense in-order PE
stream: attention starts as soon as head-pair 0's q/k m-tile is
projected, and all remaining projection work (qk m-tiles 1-3, v per
pair, out-projection) is held in a filler queue that is drained
between scores/AV groups.  The PE never waits on the ACT: every
dependency point has independent matmul work queued behind it.
ib=1 (queries 1024-2047) is processed first so the out-projection of
those rows becomes filler for the lighter ib=0 phase.

Layout notes: qT/kT = w.T @ xT (transposed acts, no on-device
transposes), v natural [s, h, 65] with a ones column per head so the
AV matmul also yields the softmax denominator; scoresT[j,i] per head
with K=64 — the two heads of an m-tile sit on partitions 0:64/64:128
so their scores matmuls land on different PE row-groups and run
concurrently (2x); max-free softmax on ScalarE (|logit| < ~7); causal
handled by narrowing to [c0, IB) plus one 128x128 triangular mask on
diagonal blocks; weights arrive as host-packed single-DMA tiles.
"""

import sys

if "/opt/trn_rl_repo" not in sys.path:
    sys.path.insert(0, "/opt/trn_rl_repo")

from collections import deque

import numpy as np
import ml_dtypes

B, S, D = 4, 2048, 1024
H, DH = 16, 64
G = 2                # tensor-parallel head groups
HPG = H // G         # heads per group (8)
CG = HPG * DH        # channel cols per group (512)
N_CORES = 8
BF16 = ml_dtypes.bfloat16

KT = D // 128        # 8 contraction k-tiles for the projections
IB = 1024            # i-block (query positions per attention block)
NIB = S // IB        # 2

_cache = {}


def _build_program():
    import concourse.tile as tile
    from concourse import bacc, mybir

    f32 = mybir.dt.float32
    bf16 = mybir.dt.bfloat16
    Exp = mybir.ActivationFunctionType.Exp
    Ident = mybir.ActivationFunctionType.Identity

    nc = bacc.Bacc("TRN2", target_bir_lowering=False, debug=False,
                   num_devices=N_CORES)

    xT_d = nc.dram_tensor("xT", [D, S], bf16, kind="ExternalInput").ap()
    # m-major pack: cols m*1024 + k*128 hold w[k-tile rows, m-tile cols]
    wq_d = nc.dram_tensor("wq", [128, 4096], bf16, kind="ExternalInput").ap()
    wk_d = nc.dram_tensor("wk", [128, 4096], bf16, kind="ExternalInput").ap()
    # k-major pack: cols k*512 hold w[k-tile rows, :]
    wv_d = nc.dram_tensor("wv", [128, 4096], bf16, kind="ExternalInput").ap()
    wo_d = nc.dram_tensor("wo", [128, 4096], bf16, kind="ExternalInput").ap()
    bqk_d = nc.dram_tensor("bqk", [128, 8], f32, kind="ExternalInput").ap()
    bv_d = nc.dram_tensor("bv", [1, CG], bf16, kind="ExternalInput").ap()
    tri_d = nc.dram_tensor("tri", [128, 128], bf16, kind="ExternalInput").ap()
    y_d = nc.dram_tensor("y", [S, D], f32, kind="ExternalOutput").ap()

    with tile.TileContext(nc) as tc:
        with (
            tc.tile_pool(name="consts", bufs=1) as cpool,
            tc.tile_pool(name="acts", bufs=1) as apool,
            tc.tile_pool(name="exps", bufs=12) as epool,
            tc.tile_pool(name="small", bufs=4) as spool,
            tc.tile_pool(name="rbc", bufs=2) as rpool,
            tc.tile_pool(name="ystage", bufs=3) as ypool,
            tc.tile_pool(name="psum_s", bufs=2, space="PSUM") as sp,
            tc.tile_pool(name="psum_av", bufs=2, space="PSUM") as avp,
        ):
            # ---- constants: few large DMAs, gating tiles first ----
            wq_sb = cpool.tile([128, 4096], bf16, tag="wq", name="wq_sb")
            wk_sb = cpool.tile([128, 4096], bf16, tag="wk", name="wk_sb")
            wv_sb = cpool.tile([128, 4096], bf16, tag="wv", name="wv_sb")
            wo_sb = cpool.tile([128, 4096], bf16, tag="wo", name="wo_sb")
            bqk = cpool.tile([128, 8], f32, tag="bqk", name="bqk")
            bv_row = cpool.tile([1, CG], bf16, tag="bv", name="bv_row")
            tri = cpool.tile([128, 128], bf16, tag="tri", name="tri")
            xt = [cpool.tile([128, S], bf16, tag=f"xt{k}", name=f"xt{k}")
                  for k in range(KT)]

            nc.sync.dma_start(xt[0][:], xT_d[0:128, :])
            nc.sync.dma_start(wq_sb[:, 0:1024], wq_d[:, 0:1024])
            nc.sync.dma_start(wk_sb[:, 0:1024], wk_d[:, 0:1024])
            nc.sync.dma_start(xt[1][:], xT_d[128:256, :])
            nc.sync.dma_start(bqk[:], bqk_d[:])
            for k in range(2, KT):
                nc.sync.dma_start(xt[k][:], xT_d[k * 128:(k + 1) * 128, :])
            nc.sync.dma_start(tri[:], tri_d[:])
            nc.sync.dma_start(bv_row[:], bv_d[:])
            nc.sync.dma_start(wv_sb[:], wv_d[:])
            nc.sync.dma_start(wq_sb[:, 1024:4096], wq_d[:, 1024:4096])
            nc.sync.dma_start(wk_sb[:, 1024:4096], wk_d[:, 1024:4096])
            nc.sync.dma_start(wo_sb[:], wo_d[:])

            ones_row = cpool.tile([1, 128], bf16, tag="ones", name="ones_row")
            nc.gpsimd.memset(ones_row[:], 1.0)

            # ---- persistent activations ----
            qT = [apool.tile([128, S], bf16, tag=f"qT{m}", name=f"qT{m}")
                  for m in range(CG // 128)]
            kTt = [apool.tile([128, S], bf16, tag=f"kT{m}", name=f"kT{m}")
                   for m in range(CG // 128)]
            vst = [apool.tile([128, HPG, DH + 1], bf16, tag=f"v{m}",
                              name=f"v{m}")
                   for m in range(S // 128)]
            aoT = [apool.tile([128, S], bf16, tag=f"aoT{m}", name=f"aoT{m}")
                   for m in range(CG // 128)]

            for st in range(S // 128):
                nc.gpsimd.memset(vst[st][:, :, DH:DH + 1], 1.0)

            # ---- work units ----
            def qk_unit(which, mi, n):
                # one psum-group of the q/k projection: out tile
                # qT/kT[mi][:, n*512:(n+1)*512]; bias added during the
                # PSUM->SBUF copy (q on ACT, k on DVE to split the load).
                wsb, out, bcol = ((wq_sb, qT, mi) if which == "q"
                                  else (wk_sb, kTt, 4 + mi))
                ps = sp.tile([128, 512], f32, tag="ps",
                             name=f"qk{which}{mi}_{n}")
                for k in range(KT):
                    nc.tensor.matmul(
                        ps[:],
                        wsb[:, mi * 1024 + k * 128:mi * 1024 + (k + 1) * 128],
                        xt[k][:, n * 512:(n + 1) * 512],
                        start=(k == 0), stop=(k == KT - 1))
                # q-bias on ACT (idle in the projection-heavy stretches),
                # k-bias on DVE: splits the PSUM-evacuation load
                dst = out[mi][:, n * 512:(n + 1) * 512]
                if which == "q":
                    nc.scalar.activation(dst, ps[:], Ident,
                                         bias=bqk[:, bcol:bcol + 1])
                else:
                    nc.vector.tensor_scalar_add(dst, ps[:],
                                                bqk[:, bcol:bcol + 1])
                return KT

            def v_unit(st):
                # v rows [st], all 8 heads (N=512 keeps the PE streaming
                # ahead of LDWEIGHTS); bias via K=1 ones x bv matmul; one
                # strided copy into the 65-col-per-head layout.
                ps = sp.tile([128, HPG, DH], f32, tag="ps", name=f"v{st}")
                for k in range(KT):
                    nc.tensor.matmul(
                        ps[:], xt[k][:, st * 128:(st + 1) * 128],
                        wv_sb[:, k * 512:(k + 1) * 512],
                        start=(k == 0), stop=False)
                nc.tensor.matmul(ps[:], ones_row[:], bv_row[:],
                                 start=False, stop=True)
                nc.scalar.activation(vst[st][:, :, 0:DH], ps[:], Ident)
                return KT + 1

            def out_unit(st):
                # out-projection for 128 query rows: y[st] = aoT[:, st].T @ wo
                ys = ypool.tile([128, 1024], f32, tag="ys", name=f"ys{st}")
                for n in range(2):
                    ps = sp.tile([128, 512], f32, tag="ps",
                                 name=f"yps{st}_{n}")
                    for k in range(CG // 128):
                        nc.tensor.matmul(
                            ps[:],
                            aoT[k][:, st * 128:(st + 1) * 128],
                            wo_sb[:, k * 1024 + n * 512:k * 1024 + (n + 1) * 512],
                            start=(k == 0), stop=(k == CG // 128 - 1))
                    nc.vector.tensor_copy(ys[:, n * 512:(n + 1) * 512], ps[:])
                nc.sync.dma_start(y_d[st * 128:(st + 1) * 128, :], ys[:])
                return 2 * (CG // 128)

            # filler queue: (key, fn) in ration order — each attention pair
            # is budgeted (allowance) the units the NEXT pair needs, so
            # filler spreads across the pair's j-tile groups instead of
            # bunching into a dense run at a pair boundary (bunching starves
            # the PE of inter-group work and lets HAM re-throttle it).
            # drain(key) force-issues everything up to and including key —
            # required when a later attention op depends on a unit's output:
            # the PE queue is in-order, so a dependency on a not-yet-issued
            # unit would deadlock (its matmuls would sit behind the stalled
            # consumer).
            fill = deque()
            issued = set()

            def pump(n_units):
                while n_units > 0 and fill:
                    k, fn = fill.popleft()
                    issued.add(k)
                    fn()
                    n_units -= 1

            def drain(key):
                if key in issued:
                    return
                while fill:
                    k, fn = fill.popleft()
                    issued.add(k)
                    fn()
                    if k == key:
                        return

            def qk_key(which, mi, n):
                return f"qk{which}{mi}_{n}"

            def add_qk(which, mi, n):
                fill.append((qk_key(which, mi, n),
                             lambda: qk_unit(which, mi, n)))

            # ---- upfront: the m0 n0/n1 halves gate pair-0 ib0 scores ----
            for n in (0, 1):
                qk_unit("q", 0, n)
                qk_unit("k", 0, n)

            # ib0-phase filler: v rows 0-7 (drained just-in-time by the
            # AVs) interleaved with the n0/n1 projection halves of pairs
            # 1-3 (force-drained before each pair's scores).
            qk01 = [(w, mi, n) for mi in (1, 2, 3) for n in (0, 1)
                    for w in ("q", "k")]
            for st in range(8):
                fill.append((f"v_{st}", lambda st=st: v_unit(st)))
                for u in qk01[2 * st:2 * st + 2]:
                    add_qk(*u)

            # ---- attention ----
            def norm_half(av, p, sub, ib, hf):
                # normalize one 512-col half of head (2p+sub): the
                # denominator row (ones column of v) divides the AV psum
                # during the copy into aoT.  Per-half so the early half's
                # psum bank frees mid-pair — the next pair's AV allocations
                # then never wait on a just-issued norm chain.  custom-DVE
                # ops must not read PSUM: the denominator goes via SBUF.
                po = DH * sub
                base = ib * IB + 512 * hf
                dn = spool.tile([1, 512], f32, tag="den", name="dn")
                nc.vector.tensor_copy(dn[:], av[DH:DH + 1, :])
                rc = spool.tile([1, 512], f32, tag="recip", name="rc")
                nc.vector.reciprocal_approx_fast(rc[:], dn[:])
                rb = rpool.tile([DH, 512], f32, tag="rbcast", name="rb")
                nc.gpsimd.partition_broadcast(rb[:], rc[:])
                nc.vector.tensor_mul(
                    aoT[p][po:po + DH, base:base + 512],
                    av[0:DH, :], rb[:])

            def attn_pair(p, ib, allowance, on_group=None):
                njt = (ib + 1) * (IB // 128)
                dstart = njt - (IB // 128)
                last = [dstart + 3, njt - 1]
                # av accumulators per (sub, half): 1 psum bank each,
                # released at their own stop+norm (half 0 mid-pair)
                avs = [[avp.tile([DH + 1, 512], f32, tag="av", bufs=4,
                                 name=f"av{p}_{ib}_{s}_{hf}")
                        for hf in range(2)] for s in range(2)]
                pend = [deque(), deque()]
                budget = [allowance]
                for jt in range(njt + SKEW):
                    if jt < njt:
                        off = jt - dstart
                        c0 = 128 * off if off > 0 else 0
                        scps = []
                        for sub in range(2):
                            po = DH * sub
                            ps = sp.tile([128, IB], f32, tag="ps",
                                         name=f"ps{p}_{ib}_{jt}_{sub}")
                            for lo, hi in _halves(c0):
                                nc.tensor.matmul(
                                    ps[:, lo:hi],
                                    kTt[p][po:po + DH,
                                           jt * 128:(jt + 1) * 128],
                                    qT[p][po:po + DH,
                                          ib * IB + lo:ib * IB + hi],
                                    start=True, stop=True)
                            scps.append(ps)
                        for sub in range(2):
                            et = epool.tile([128, IB], bf16, tag="expT",
                                            name="et")
                            nc.scalar.activation(
                                et[:, c0:IB], scps[sub][:, c0:IB],
                                Exp, scale=float(DH) ** -0.5)
                            if jt >= dstart:
                                nc.vector.tensor_mul(
                                    et[:, c0:c0 + 128],
                                    et[:, c0:c0 + 128], tri[:])
                            pend[sub].append((jt, et, c0))
                    if jt == 8 and ib == 1:
                        # j-tiles 8-15 read the n2/n3 half of kT
                        drain(qk_key("k", p, 2))
                        drain(qk_key("k", p, 3))
                    if jt % 2 == 1 or jt >= njt:
                        if on_group is not None:
                            on_group(jt)
                        if budget[0] > 0:
                            pump(2)
                            budget[0] -= 2
                        for sub in range(2):
                            h = 2 * p + sub
                            while pend[sub] and (
                                    len(pend[sub]) > SKEW or jt >= njt):
                                jt0, et, c0 = pend[sub].popleft()
                                drain(f"v_{jt0}")
                                for lo, hi in _halves(c0):
                                    hf = hi // 512 - 1
                                    nc.tensor.matmul(
                                        avs[sub][hf][:, lo - 512 * hf:
                                                     hi - 512 * hf],
                                        vst[jt0][:, h, :],
                                        et[:, lo:hi],
                                        start=(jt0 == 0),
                                        stop=(jt0 == last[hf]))
                                    if jt0 == last[hf]:
                                        norm_half(avs[sub][hf], p, sub,
                                                  ib, hf)

            SKEW = 2
            # remaining projection halves (n2/n3, JIT-drained per ib1
            # pair) and v rows 8-15 (JIT by the ib1 AVs) are dependency-
            # safe any time — queue them now so late-ib0 pairs never run
            # out of filler.
            qk23 = [(w, mi, n) for mi in range(4) for n in (2, 3)
                    for w in ("q", "k")]
            for st in range(8, 16):
                fill.append((f"v_{st}", lambda st=st: v_unit(st)))
                for u in qk23[2 * (st - 8):2 * (st - 8) + 2]:
                    add_qk(*u)

            # ---- ib0 phase: scores/AV on queries 0-1023 over v rows 0-7,
            # with v and the projection halves as PE filler ----
            for p in range(HPG // 2):
                if p > 0:
                    drain(qk_key("q", p, 1))
                    drain(qk_key("k", p, 1))
                attn_pair(p, 0, allowance=(12 if p == 0 else 6))

            # out-projection of the now-finished ib0 query rows: must be
            # issued only after the ib0 norms above (in-order PE queue)
            for st in range(0, 8):
                fill.append((f"o{st}", lambda st=st: out_unit(st)))

            # out-projection rows 1024-1535 depend only on the ib1 half-0
            # norms (done by j-tile 11): release those units mid-pair-3 so
            # the tail shrinks to rows 1536-2047.
            o_early = {"done": False}

            def late_fill(jt):
                if jt >= 12 and not o_early["done"]:
                    o_early["done"] = True
                    for st in range(8, 12):
                        fill.append((f"o{st}", lambda st=st: out_unit(st)))

            for p in range(HPG // 2):
                drain(qk_key("q", p, 2))
                drain(qk_key("q", p, 3))
                attn_pair(p, 1, allowance=(12 if p == 0 else 7),
                          on_group=late_fill if p == 3 else None)
            while fill:
                k, fn = fill.popleft()
                issued.add(k)
                fn()
            for st in range(12, 16):
                out_unit(st)

    nc.compile()
    return nc


def _halves(c0):
    # the two 512-wide PSUM-bank column ranges, narrowed to the causally
    # valid region [c0, IB)
    for n in range(IB // 512):
        lo, hi = max(n * 512, c0), (n + 1) * 512
        if lo < hi:
            yield lo, hi


def _shard_inputs(x, w_qkv, b_qkv, w_out):
    # keep key j (partition) <= query i (free column): upper triangle
    tri = np.triu(np.ones((128, 128))).astype(BF16)
    in_maps = []
    for c in range(N_CORES):
        b, g = c // G, c % G
        sl = slice(g * CG, (g + 1) * CG)
        wq = w_qkv[:, 0 * D:1 * D][:, sl].astype(BF16)   # [1024, 512]
        wk = w_qkv[:, 1 * D:2 * D][:, sl].astype(BF16)
        wv = w_qkv[:, 2 * D:3 * D][:, sl].astype(BF16)
        wo = w_out[sl, :].astype(BF16)                   # [512, 1024]
        # m-major pack [128, 4096]: cols m*1024 + k*128
        wq_p = np.concatenate(
            [np.concatenate([wq[k * 128:(k + 1) * 128,
                                m * 128:(m + 1) * 128] for k in range(KT)],
                            axis=1) for m in range(4)], axis=1)
        wk_p = np.concatenate(
            [np.concatenate([wk[k * 128:(k + 1) * 128,
                                m * 128:(m + 1) * 128] for k in range(KT)],
                            axis=1) for m in range(4)], axis=1)
        # k-major pack [128, 4096]: cols k*512
        wv_p = np.concatenate([wv[k * 128:(k + 1) * 128, :]
                               for k in range(KT)], axis=1)
        wo_p = np.concatenate([wo[k * 128:(k + 1) * 128, :]
                               for k in range(4)], axis=1)
        bq = b_qkv[0 * D:1 * D][sl].reshape(4, 128).T    # [128, 4]
        bk = b_qkv[1 * D:2 * D][sl].reshape(4, 128).T
        in_maps.append({
            "xT": np.ascontiguousarray(x[b].T).astype(BF16),
            "wq": np.ascontiguousarray(wq_p).astype(BF16),
            "wk": np.ascontiguousarray(wk_p).astype(BF16),
            "wv": np.ascontiguousarray(wv_p).astype(BF16),
            "wo": np.ascontiguousarray(wo_p).astype(BF16),
            "bqk": np.ascontiguousarray(
                np.concatenate([bq, bk], axis=1)).astype(np.float32),
            "bv": b_qkv[2 * D:3 * D][sl].reshape(1, CG).astype(BF16),
            "tri": tri,
        })
    return in_maps


def kernel(x, w_qkv, b_qkv, w_out, b_out):
    from concourse.bass_utils import run_bass_kernel_spmd

    x = np.asarray(x, np.float32)
    w_qkv = np.asarray(w_qkv, np.float32)
    b_qkv = np.asarray(b_qkv, np.float32)
    w_out = np.asarray(w_out, np.float32)
    b_out = np.asarray(b_out, np.float32)

    if "nc" not in _cache:
        _cache["nc"] = _build_program()
    nc = _cache["nc"]

    in_maps = _shard_inputs(x, w_qkv, b_qkv, w_out)
    res = run_bass_kernel_spmd(nc, in_maps, core_ids=list(range(N_CORES)))
    _cache["last_result"] = res

    y = np.empty((B, S, D), np.float32)
    for b in range(B):
        y[b] = res.results[G * b]["y"] + res.results[G * b + 1]["y"] + b_out
    return y


# revision 35
# speedup vs baseline: 1.2642x; 1.0047x over previous
"""Trainium2 Bass kernel for causal multi-head attention (v2).

Problem: B=4, S=2048, D=1024, H=16 heads (d_head=64), fp32 I/O.
    qkv = x @ w_qkv + b_qkv ; causal softmax attention ; out @ w_out + b_out

Sharding over 8 NeuronCores: data-parallel over batch (4) x
tensor-parallel over head-groups (2 groups of 8 heads). Core c handles
batch c//2, head-group c%2. No collectives: each core returns its
partial out-projection; the host sums the two group partials per batch
and adds b_out.

v2 schedule (vs v1): the kernel is ACT(exp)-heavy in attention and
PE-heavy in the projections; v1 ran them serially, so the PE idled
waiting on exp in attention, HAM re-throttled it to half clock, and
the whole attention phase crawled.  v2 keeps one dense in-order PE
stream: attention starts as soon as head-pair 0's q/k m-tile is
projected, and all remaining projection work (qk m-tiles 1-3, v per
pair, out-projection) is held in a filler queue that is drained
between scores/AV groups.  The PE never waits on the ACT: every
dependency point has independent matmul work queued behind it.
ib=1 (queries 1024-2047) is processed first so the out-projection of
those rows becomes filler for the lighter ib=0 phase.

Layout notes: qT/kT = w.T @ xT (transposed acts, no on-device
transposes), v natural [s, h, 65] with a ones column per head so the
AV matmul also yields the softmax denominator; scoresT[j,i] per head
with K=64 — the two heads of an m-tile sit on partitions 0:64/64:128
so their scores matmuls land on different PE row-groups and run
concurrently (2x); max-free softmax on ScalarE (|logit| < ~7); causal
handled by narrowing to [c0, IB) plus one 128x128 triangular mask on
diagonal blocks; weights arrive as host-packed single-DMA tiles.
"""

import sys

if "/opt/trn_rl_repo" not in sys.path:
    sys.path.insert(0, "/opt/trn_rl_repo")

from collections import deque

import numpy as np
import ml_dtypes

B, S, D = 4, 2048, 1024
H, DH = 16, 64
G = 2                # tensor-parallel head groups
HPG = H // G         # heads per group (8)
CG = HPG * DH        # channel cols per group (512)
N_CORES = 8
BF16 = ml_dtypes.bfloat16

KT = D // 128        # 8 contraction k-tiles for the projections
IB = 1024            # i-block (query positions per attention block)
NIB = S // IB        # 2

_cache = {}


def _build_program():
    import concourse.tile as tile
    from concourse import bacc, mybir

    f32 = mybir.dt.float32
    bf16 = mybir.dt.bfloat16
    Exp = mybir.ActivationFunctionType.Exp
    Ident = mybir.ActivationFunctionType.Identity

    nc = bacc.Bacc("TRN2", target_bir_lowering=False, debug=False,
                   num_devices=N_CORES)

    xT_d = nc.dram_tensor("xT", [D, S], bf16, kind="ExternalInput").ap()
    # m-major pack: cols m*1024 + k*128 hold w[k-tile rows, m-tile cols]
    wq_d = nc.dram_tensor("wq", [128, 4096], bf16, kind="ExternalInput").ap()
    wk_d = nc.dram_tensor("wk", [128, 4096], bf16, kind="ExternalInput").ap()
    # k-major pack: cols k*512 hold w[k-tile rows, :]
    wv_d = nc.dram_tensor("wv", [128, 4096], bf16, kind="ExternalInput").ap()
    wo_d = nc.dram_tensor("wo", [128, 4096], bf16, kind="ExternalInput").ap()
    bqk_d = nc.dram_tensor("bqk", [128, 8], f32, kind="ExternalInput").ap()
    bv_d = nc.dram_tensor("bv", [1, CG], bf16, kind="ExternalInput").ap()
    tri_d = nc.dram_tensor("tri", [128, 128], bf16, kind="ExternalInput").ap()
    y_d = nc.dram_tensor("y", [S, D], f32, kind="ExternalOutput").ap()

    with tile.TileContext(nc) as tc:
        with (
            tc.tile_pool(name="consts", bufs=1) as cpool,
            tc.tile_pool(name="acts", bufs=1) as apool,
            tc.tile_pool(name="exps", bufs=12) as epool,
            tc.tile_pool(name="small", bufs=4) as spool,
            tc.tile_pool(name="rbc", bufs=2) as rpool,
            tc.tile_pool(name="ystage", bufs=3) as ypool,
            tc.tile_pool(name="psum_s", bufs=2, space="PSUM") as sp,
            tc.tile_pool(name="psum_av", bufs=2, space="PSUM") as avp,
        ):
            # ---- constants: few large DMAs, gating tiles first ----
            wq_sb = cpool.tile([128, 4096], bf16, tag="wq", name="wq_sb")
            wk_sb = cpool.tile([128, 4096], bf16, tag="wk", name="wk_sb")
            wv_sb = cpool.tile([128, 4096], bf16, tag="wv", name="wv_sb")
            wo_sb = cpool.tile([128, 4096], bf16, tag="wo", name="wo_sb")
            bqk = cpool.tile([128, 8], f32, tag="bqk", name="bqk")
            bv_row = cpool.tile([1, CG], bf16, tag="bv", name="bv_row")
            tri = cpool.tile([128, 128], bf16, tag="tri", name="tri")
            xt = [cpool.tile([128, S], bf16, tag=f"xt{k}", name=f"xt{k}")
                  for k in range(KT)]

            nc.sync.dma_start(xt[0][:], xT_d[0:128, :])
            nc.sync.dma_start(wq_sb[:, 0:1024], wq_d[:, 0:1024])
            nc.sync.dma_start(wk_sb[:, 0:1024], wk_d[:, 0:1024])
            nc.sync.dma_start(xt[1][:], xT_d[128:256, :])
            nc.sync.dma_start(bqk[:], bqk_d[:])
            for k in range(2, KT):
                nc.sync.dma_start(xt[k][:], xT_d[k * 128:(k + 1) * 128, :])
            nc.sync.dma_start(tri[:], tri_d[:])
            nc.sync.dma_start(bv_row[:], bv_d[:])
            nc.sync.dma_start(wv_sb[:], wv_d[:])
            nc.sync.dma_start(wq_sb[:, 1024:4096], wq_d[:, 1024:4096])
            nc.sync.dma_start(wk_sb[:, 1024:4096], wk_d[:, 1024:4096])
            nc.sync.dma_start(wo_sb[:], wo_d[:])

            ones_row = cpool.tile([1, 128], bf16, tag="ones", name="ones_row")
            nc.gpsimd.memset(ones_row[:], 1.0)

            # ---- persistent activations ----
            qT = [apool.tile([128, S], bf16, tag=f"qT{m}", name=f"qT{m}")
                  for m in range(CG // 128)]
            kTt = [apool.tile([128, S], bf16, tag=f"kT{m}", name=f"kT{m}")
                   for m in range(CG // 128)]
            vst = [apool.tile([128, HPG, DH + 1], bf16, tag=f"v{m}",
                              name=f"v{m}")
                   for m in range(S // 128)]
            aoT = [apool.tile([128, S], bf16, tag=f"aoT{m}", name=f"aoT{m}")
                   for m in range(CG // 128)]

            for st in range(S // 128):
                nc.gpsimd.memset(vst[st][:, :, DH:DH + 1], 1.0)

            # ---- work units ----
            def qk_unit(which, mi, n):
                # one psum-group of the q/k projection: out tile
                # qT/kT[mi][:, n*512:(n+1)*512]; bias added during the
                # PSUM->SBUF copy (q on ACT, k on DVE to split the load).
                wsb, out, bcol = ((wq_sb, qT, mi) if which == "q"
                                  else (wk_sb, kTt, 4 + mi))
                ps = sp.tile([128, 512], f32, tag="ps",
                             name=f"qk{which}{mi}_{n}")
                for k in range(KT):
                    nc.tensor.matmul(
                        ps[:],
                        wsb[:, mi * 1024 + k * 128:mi * 1024 + (k + 1) * 128],
                        xt[k][:, n * 512:(n + 1) * 512],
                        start=(k == 0), stop=(k == KT - 1))
                # q-bias on ACT (idle in the projection-heavy stretches),
                # k-bias on DVE: splits the PSUM-evacuation load
                dst = out[mi][:, n * 512:(n + 1) * 512]
                if which == "q":
                    nc.scalar.activation(dst, ps[:], Ident,
                                         bias=bqk[:, bcol:bcol + 1])
                else:
                    nc.vector.tensor_scalar_add(dst, ps[:],
                                                bqk[:, bcol:bcol + 1])
                return KT

            def v_unit(st):
                # v rows [st], all 8 heads (N=512 keeps the PE streaming
                # ahead of LDWEIGHTS); bias via K=1 ones x bv matmul; one
                # strided copy into the 65-col-per-head layout.
                ps = sp.tile([128, HPG, DH], f32, tag="ps", name=f"v{st}")
                for k in range(KT):
                    nc.tensor.matmul(
                        ps[:], xt[k][:, st * 128:(st + 1) * 128],
                        wv_sb[:, k * 512:(k + 1) * 512],
                        start=(k == 0), stop=False)
                nc.tensor.matmul(ps[:], ones_row[:], bv_row[:],
                                 start=False, stop=True)
                nc.scalar.activation(vst[st][:, :, 0:DH], ps[:], Ident)
                return KT + 1

            def out_unit(st):
                # out-projection for 128 query rows: y[st] = aoT[:, st].T @ wo
                ys = ypool.tile([128, 1024], f32, tag="ys", name=f"ys{st}")
                for n in range(2):
                    ps = sp.tile([128, 512], f32, tag="ps",
                                 name=f"yps{st}_{n}")
                    for k in range(CG // 128):
                        nc.tensor.matmul(
                            ps[:],
                            aoT[k][:, st * 128:(st + 1) * 128],
                            wo_sb[:, k * 1024 + n * 512:k * 1024 + (n + 1) * 512],
                            start=(k == 0), stop=(k == CG // 128 - 1))
                    nc.vector.tensor_copy(ys[:, n * 512:(n + 1) * 512], ps[:])
                nc.sync.dma_start(y_d[st * 128:(st + 1) * 128, :], ys[:])
                return 2 * (CG // 128)

            # filler queue: (key, fn) in ration order — each attention pair
            # is budgeted (allowance) the units the NEXT pair needs, so
            # filler spreads across the pair's j-tile groups instead of
            # bunching into a dense run at a pair boundary (bunching starves
            # the PE of inter-group work and lets HAM re-throttle it).
            # drain(key) force-issues everything up to and including key —
            # required when a later attention op depends on a unit's output:
            # the PE queue is in-order, so a dependency on a not-yet-issued
            # unit would deadlock (its matmuls would sit behind the stalled
            # consumer).
            fill = deque()
            issued = set()

            def pump(n_units):
                while n_units > 0 and fill:
                    k, fn = fill.popleft()
                    issued.add(k)
                    fn()
                    n_units -= 1

            def drain(key):
                if key in issued:
                    return
                while fill:
                    k, fn = fill.popleft()
                    issued.add(k)
                    fn()
                    if k == key:
                        return

            def qk_key(which, mi, n):
                return f"qk{which}{mi}_{n}"

            def add_qk(which, mi, n):
                fill.append((qk_key(which, mi, n),
                             lambda: qk_unit(which, mi, n)))

            # ---- upfront: the m0 n0/n1 halves gate pair-0 ib0 scores ----
            for n in (0, 1):
                qk_unit("q", 0, n)
                qk_unit("k", 0, n)

            # ib0-phase filler: v rows 0-7 (drained just-in-time by the
            # AVs) interleaved with the n0/n1 projection halves of pairs
            # 1-3 (force-drained before each pair's scores).
            qk01 = [(w, mi, n) for mi in (1, 2, 3) for n in (0, 1)
                    for w in ("q", "k")]
            for st in range(8):
                fill.append((f"v_{st}", lambda st=st: v_unit(st)))
                for u in qk01[2 * st:2 * st + 2]:
                    add_qk(*u)

            # ---- attention ----
            def norm_half(av, p, sub, ib, hf):
                # normalize one 512-col half of head (2p+sub): the
                # denominator row (ones column of v) divides the AV psum
                # during the copy into aoT.  Per-half so the early half's
                # psum bank frees mid-pair — the next pair's AV allocations
                # then never wait on a just-issued norm chain.  custom-DVE
                # ops must not read PSUM: the denominator goes via SBUF.
                po = DH * sub
                base = ib * IB + 512 * hf
                dn = spool.tile([1, 512], f32, tag="den", name="dn")
                nc.vector.tensor_copy(dn[:], av[DH:DH + 1, :])
                rc = spool.tile([1, 512], f32, tag="recip", name="rc")
                nc.vector.reciprocal_approx_fast(rc[:], dn[:])
                rb = rpool.tile([DH, 512], f32, tag="rbcast", name="rb")
                nc.gpsimd.partition_broadcast(rb[:], rc[:])
                nc.vector.tensor_mul(
                    aoT[p][po:po + DH, base:base + 512],
                    av[0:DH, :], rb[:])

            # pending AV closures, carried ACROSS pair boundaries: the
            # last j-tiles of a pair are consumed during the next pair's
            # early groups, so the exp->AV skew never collapses to zero
            # (a zero-skew pair tail makes the PE wait on a just-issued
            # exp at every boundary and HAM re-throttles it).
            avq = deque()

            def flush_avq(n_keep):
                while len(avq) > n_keep:
                    avq.popleft()()

            def attn_pair(p, ib, allowance, on_group=None):
                njt = (ib + 1) * (IB // 128)
                dstart = njt - (IB // 128)
                last = [dstart + 3, njt - 1]
                # av accumulators per (sub, half): 1 psum bank each,
                # released at their own stop+norm (half 0 mid-pair)
                avs = [[avp.tile([DH + 1, 512], f32, tag="av", bufs=4,
                                 name=f"av{p}_{ib}_{s}_{hf}")
                        for hf in range(2)] for s in range(2)]
                budget = [allowance]

                def av_op(sub, jt0, et, c0):
                    h = 2 * p + sub
                    drain(f"v_{jt0}")
                    for lo, hi in _halves(c0):
                        hf = hi // 512 - 1
                        nc.tensor.matmul(
                            avs[sub][hf][:, lo - 512 * hf:hi - 512 * hf],
                            vst[jt0][:, h, :],
                            et[:, lo:hi],
                            start=(jt0 == 0),
                            stop=(jt0 == last[hf]))
                        if jt0 == last[hf]:
                            norm_half(avs[sub][hf], p, sub, ib, hf)

                for jt in range(njt):
                    off = jt - dstart
                    c0 = 128 * off if off > 0 else 0
                    scps = []
                    for sub in range(2):
                        po = DH * sub
                        ps = sp.tile([128, IB], f32, tag="ps",
                                     name=f"ps{p}_{ib}_{jt}_{sub}")
                        for lo, hi in _halves(c0):
                            nc.tensor.matmul(
                                ps[:, lo:hi],
                                kTt[p][po:po + DH,
                                       jt * 128:(jt + 1) * 128],
                                qT[p][po:po + DH,
                                      ib * IB + lo:ib * IB + hi],
                                start=True, stop=True)
                        scps.append(ps)
                    for sub in range(2):
                        et = epool.tile([128, IB], bf16, tag="expT",
                                        name="et")
                        nc.scalar.activation(
                            et[:, c0:IB], scps[sub][:, c0:IB],
                            Exp, scale=float(DH) ** -0.5)
                        if jt >= dstart:
                            nc.vector.tensor_mul(
                                et[:, c0:c0 + 128],
                                et[:, c0:c0 + 128], tri[:])
                        avq.append(
                            lambda sub=sub, jt=jt, et=et, c0=c0:
                            av_op(sub, jt, et, c0))
                    if jt == 8 and ib == 1:
                        # j-tiles 8-15 read the n2/n3 half of kT
                        drain(qk_key("k", p, 2))
                        drain(qk_key("k", p, 3))
                    if jt % 2 == 1:
                        if on_group is not None:
                            on_group(jt)
                        if budget[0] > 0:
                            pump(2)
                            budget[0] -= 2
                        flush_avq(2 * SKEW)

            SKEW = 2
            # remaining projection halves (n2/n3, JIT-drained per ib1
            # pair) and v rows 8-15 (JIT by the ib1 AVs) are dependency-
            # safe any time — queue them now so late-ib0 pairs never run
            # out of filler.
            qk23 = [(w, mi, n) for mi in range(4) for n in (2, 3)
                    for w in ("q", "k")]
            for st in range(8, 16):
                fill.append((f"v_{st}", lambda st=st: v_unit(st)))
                for u in qk23[2 * (st - 8):2 * (st - 8) + 2]:
                    add_qk(*u)

            # ---- ib0 phase: scores/AV on queries 0-1023 over v rows 0-7,
            # with v and the projection halves as PE filler ----
            for p in range(HPG // 2):
                if p > 0:
                    drain(qk_key("q", p, 1))
                    drain(qk_key("k", p, 1))
                attn_pair(p, 0, allowance=(12 if p == 0 else 6))

            # out-projection of the now-finished ib0 query rows: must be
            # issued only after the ib0 norms above (in-order PE queue)
            for st in range(0, 8):
                fill.append((f"o{st}", lambda st=st: out_unit(st)))

            # out-projection rows 1024-1535 depend only on the ib1 half-0
            # norms (done by j-tile 11): release those units mid-pair-3 so
            # the tail shrinks to rows 1536-2047.
            o_early = {"done": False}

            def late_fill(jt):
                if jt >= 12 and not o_early["done"]:
                    o_early["done"] = True
                    for st in range(8, 12):
                        fill.append((f"o{st}", lambda st=st: out_unit(st)))

            for p in range(HPG // 2):
                drain(qk_key("q", p, 2))
                drain(qk_key("q", p, 3))
                attn_pair(p, 1, allowance=(12 if p == 0 else 7),
                          on_group=late_fill if p == 3 else None)
            flush_avq(0)
            while fill:
                k, fn = fill.popleft()
                issued.add(k)
                fn()
            for st in range(12, 16):
                out_unit(st)

    nc.compile()
    return nc


def _halves(c0):
    # the two 512-wide PSUM-bank column ranges, narrowed to the causally
    # valid region [c0, IB)
    for n in range(IB // 512):
        lo, hi = max(n * 512, c0), (n + 1) * 512
        if lo < hi:
            yield lo, hi


def _shard_inputs(x, w_qkv, b_qkv, w_out):
    # keep key j (partition) <= query i (free column): upper triangle
    tri = np.triu(np.ones((128, 128))).astype(BF16)
    in_maps = []
    for c in range(N_CORES):
        b, g = c // G, c % G
        sl = slice(g * CG, (g + 1) * CG)
        wq = w_qkv[:, 0 * D:1 * D][:, sl].astype(BF16)   # [1024, 512]
        wk = w_qkv[:, 1 * D:2 * D][:, sl].astype(BF16)
        wv = w_qkv[:, 2 * D:3 * D][:, sl].astype(BF16)
        wo = w_out[sl, :].astype(BF16)                   # [512, 1024]
        # m-major pack [128, 4096]: cols m*1024 + k*128
        wq_p = np.concatenate(
            [np.concatenate([wq[k * 128:(k + 1) * 128,
                                m * 128:(m + 1) * 128] for k in range(KT)],
                            axis=1) for m in range(4)], axis=1)
        wk_p = np.concatenate(
            [np.concatenate([wk[k * 128:(k + 1) * 128,
                                m * 128:(m + 1) * 128] for k in range(KT)],
                            axis=1) for m in range(4)], axis=1)
        # k-major pack [128, 4096]: cols k*512
        wv_p = np.concatenate([wv[k * 128:(k + 1) * 128, :]
                               for k in range(KT)], axis=1)
        wo_p = np.concatenate([wo[k * 128:(k + 1) * 128, :]
                               for k in range(4)], axis=1)
        bq = b_qkv[0 * D:1 * D][sl].reshape(4, 128).T    # [128, 4]
        bk = b_qkv[1 * D:2 * D][sl].reshape(4, 128).T
        in_maps.append({
            "xT": np.ascontiguousarray(x[b].T).astype(BF16),
            "wq": np.ascontiguousarray(wq_p).astype(BF16),
            "wk": np.ascontiguousarray(wk_p).astype(BF16),
            "wv": np.ascontiguousarray(wv_p).astype(BF16),
            "wo": np.ascontiguousarray(wo_p).astype(BF16),
            "bqk": np.ascontiguousarray(
                np.concatenate([bq, bk], axis=1)).astype(np.float32),
            "bv": b_qkv[2 * D:3 * D][sl].reshape(1, CG).astype(BF16),
            "tri": tri,
        })
    return in_maps


def kernel(x, w_qkv, b_qkv, w_out, b_out):
    from concourse.bass_utils import run_bass_kernel_spmd

    x = np.asarray(x, np.float32)
    w_qkv = np.asarray(w_qkv, np.float32)
    b_qkv = np.asarray(b_qkv, np.float32)
    w_out = np.asarray(w_out, np.float32)
    b_out = np.asarray(b_out, np.float32)

    if "nc" not in _cache:
        _cache["nc"] = _build_program()
    nc = _cache["nc"]

    in_maps = _shard_inputs(x, w_qkv, b_qkv, w_out)
    res = run_bass_kernel_spmd(nc, in_maps, core_ids=list(range(N_CORES)))
    _cache["last_result"] = res

    y = np.empty((B, S, D), np.float32)
    for b in range(B):
        y[b] = res.results[G * b]["y"] + res.results[G * b + 1]["y"] + b_out
    return y
